# revision 4
# baseline (speedup 1.0000x reference)
"""Causal multi-head self-attention on 8 Trainium2 NeuronCores.

Sharding: core = (batch b, head-group g).  B=4 batches x 2 groups of 8 heads
= 8 cores.  Each core computes Q/K/V projections for its 8 heads, causal
attention, and a partial output projection (row-shard of WO); the host sums
the two partials per batch (the tensor-parallel all-reduce, done at gather).

Per-core device pipeline (all matmuls in float32r = full fp32 precision via
the PE's 2-pass mode, 1 cycle/row at N>=256):
  stage 1: QT[d',s], KT[d',s] (transposed) and V[s,d'] (natural) projections
           from host-pre-transposed xT and weight shards.
  stage 2: per (head-pair, 512-wide q-block): scoresT[k,q] per 128-wide
           k-chunk, causal mask add on the single partial 128-col stripe of
           diagonal chunks, exp over the allowed column range only (no max
           subtraction -- scores are O(5) so exp is safe in fp32), attn@V
           with a ones column appended to V so PSUM row 64 accumulates the
           softmax denominator.  Denominators are gathered to a [8, 512]
           tile per pair so the reciprocal runs on 8 partitions at once;
           normalization is an in-place multiply on the unnormalized AT.
  stage 3: output projection AT^T @ woT -> out rows, DMA to DRAM.
"""

import os
import numpy as np

B, S, D = 4, 2048, 1024
H_TOTAL, DK = 16, 64
G = 2          # head groups (cores per batch)
HG = 8         # heads per core
DG = 512       # head dims per core
CO = 8         # contraction chunks of 128 over D
SBLK = 4       # 512-wide s blocks
QB = 4         # 512-wide q blocks
NEG = -1e9

_BUILD_CACHE = {}


def _build():
    if "nc" in _BUILD_CACHE:
        return _BUILD_CACHE["nc"]

    import concourse.bacc as bacc
    import concourse.mybir as mybir
    import concourse.tile as tile

    f32 = mybir.dt.float32
    f32r = mybir.dt.float32r
    AF = mybir.ActivationFunctionType
    ADD = mybir.AluOpType.add
    MULT = mybir.AluOpType.mult

    nc = bacc.Bacc("TRN2", target_bir_lowering=False)
    xT_d = nc.dram_tensor("xT", [D, S], f32, kind="ExternalInput")
    wq_d = nc.dram_tensor("wqT", [D, DG], f32, kind="ExternalInput")
    wk_d = nc.dram_tensor("wkT", [D, DG], f32, kind="ExternalInput")
    wv_d = nc.dram_tensor("wvT", [D, DG], f32, kind="ExternalInput")
    wo_d = nc.dram_tensor("woT", [DG, D], f32, kind="ExternalInput")
    mask_d = nc.dram_tensor("mask", [128, 128], f32, kind="ExternalInput")
    ones_d = nc.dram_tensor("onesb", [128, 128], f32, kind="ExternalInput")
    out_d = nc.dram_tensor("out", [S, D], f32, kind="ExternalOutput")

    with tile.TileContext(nc) as tc:
        with tc.tile_pool(name="persist", bufs=1) as pp:
            QT = pp.tile([128, 4, S], f32r, tag="QT")
            KT = pp.tile([128, 4, S], f32r, tag="KT")
            V = pp.tile([128, 16, HG, DK + 1], f32r, tag="V")
            maskb = pp.tile([128, 128], f32, tag="maskb")
            onesb = pp.tile([128, 128], f32, tag="onesb")
            ones_r = pp.tile([1, 64], f32r, tag="ones_r")
            nc.sync.dma_start(maskb[:], mask_d[:, :])
            nc.sync.dma_start(onesb[:], ones_d[:, :])
            nc.sync.dma_start(ones_r[:], ones_d[0:1, 0:64].bitcast(f32r))
            # ones column of V (f32 -> f32r rounding copy)
            nc.vector.tensor_copy(
                V[:, :, :, DK : DK + 1],
                onesb[:, 0:128].rearrange("p (so h) -> p so h", so=16)[:, :, :, None],
            )

            # ---------------- stage 1: projections ----------------
            with (
                tc.tile_pool(name="stage1", bufs=1) as s1p,
                tc.tile_pool(name="ps1", bufs=1, space="PSUM") as ps1,
            ):
                wq = s1p.tile([128, CO, DG], f32r, tag="wq")
                wk = s1p.tile([128, CO, DG], f32r, tag="wk")
                wv = s1p.tile([128, CO, DG], f32r, tag="wv")
                nc.sync.dma_start(
                    wq, wq_d[:, :].rearrange("(co ci) d -> ci co d", ci=128).bitcast(f32r)
                )
                nc.sync.dma_start(
                    wk, wk_d[:, :].rearrange("(co ci) d -> ci co d", ci=128).bitcast(f32r)
                )
                nc.sync.dma_start(
                    wv, wv_d[:, :].rearrange("(co ci) d -> ci co d", ci=128).bitcast(f32r)
                )
                for sb in range(SBLK):
                    xt = s1p.tile([128, CO, 512], f32r, tag="xt", bufs=2)
                    nc.sync.dma_start(
                        xt,
                        xT_d[:, sb * 512 : (sb + 1) * 512]
                        .rearrange("(co ci) s -> ci co s", ci=128)
                        .bitcast(f32r),
                    )
                    ssl = slice(sb * 512, (sb + 1) * 512)
                    for do in range(4):
                        dsl = slice(do * 128, (do + 1) * 128)
                        pq = ps1.tile([128, 512], f32, tag="proj", bufs=4, name=f"pq{sb}{do}")
                        for co in range(CO):
                            nc.tensor.matmul(
                                pq, wq[:, co, dsl], xt[:, co, :],
                                start=(co == 0), stop=(co == CO - 1),
                            )
                        nc.any.tensor_copy(QT[:, do, ssl], pq[:])
                        pk = ps1.tile([128, 512], f32, tag="proj", bufs=4, name=f"pk{sb}{do}")
                        for co in range(CO):
                            nc.tensor.matmul(
                                pk, wk[:, co, dsl], xt[:, co, :],
                                start=(co == 0), stop=(co == CO - 1),
                            )
                        nc.any.tensor_copy(KT[:, do, ssl], pk[:])
                    for so in range(4):
                        pv = ps1.tile([128, 512], f32, tag="proj", bufs=4, name=f"pv{sb}{so}")
                        for co in range(CO):
                            nc.tensor.matmul(
                                pv, xt[:, co, so * 128 : (so + 1) * 128], wv[:, co, :],
                                start=(co == 0), stop=(co == CO - 1),
                            )
                        nc.any.tensor_copy(
                            V[:, sb * 4 + so, :, 0:DK],
                            pv[:].rearrange("p (h d) -> p h d", h=HG),
                        )

            # ---------------- stage 2: attention ----------------
            with tc.tile_pool(name="atp", bufs=1) as atp:
              AT = atp.tile([128, 4, S], f32r, tag="AT")
              with (
                tc.tile_pool(name="stage2", bufs=1) as s2p,
                tc.tile_pool(name="ps2", bufs=1, space="PSUM") as ps2,
              ):
                for pair in range(4):
                    heads = (2 * pair, 2 * pair + 1)
                    # denominator gather tiles; rows at 32-aligned partitions
                    # (SBUF partition offsets must be 32-aligned).  memset to
                    # 1.0 so the batched reciprocal sees no garbage lanes.
                    sums_a = s2p.tile([128, 512], f32, tag="sums_a", bufs=2, name=f"sums_a{pair}")
                    sums_b = s2p.tile([128, 512], f32, tag="sums_b", bufs=2, name=f"sums_b{pair}")
                    nc.any.memset(sums_a[:], 1.0)
                    nc.any.memset(sums_b[:], 1.0)
                    sums_of = {heads[0]: sums_a, heads[1]: sums_b}
                    for qb in range(QB):
                        qsl = slice(qb * 512, (qb + 1) * 512)
                        ovs = {}
                        for h in heads:
                            ov = ps2.tile(
                                [DK + 1, 512], f32, tag="ov", bufs=3, name=f"ov{h}q{qb}"
                            )
                            ovs[h] = ov
                        nkb = 4 * qb + 4
                        for kb in range(nkb):
                            ksl = slice(kb * 128, (kb + 1) * 128)
                            d = kb - 4 * qb
                            cs = 128 * d if d > 0 else 0
                            for h in heads:
                                base = 64 * (h % 2)
                                psl = slice(base, base + 64)
                                sp = ps2.tile(
                                    [128, 512], f32, tag="score", bufs=4,
                                    name=f"sp{h}q{qb}k{kb}",
                                )
                                nc.tensor.matmul(
                                    sp, KT[psl, pair, ksl], QT[psl, pair, qsl],
                                    start=True, stop=True,
                                )
                                if d >= 0:
                                    # only the 128-col stripe [128d, 128d+128)
                                    # is partially masked
                                    nc.vector.tensor_tensor(
                                        sp[:, cs : cs + 128],
                                        sp[:, cs : cs + 128],
                                        maskb[:, :],
                                        ADD,
                                    )
                                et = s2p.tile(
                                    [128, 512], f32r, tag="et", bufs=10,
                                    name=f"et{h}q{qb}k{kb}",
                                )
                                nc.scalar.activation(
                                    et[:, cs:], sp[:, cs:], AF.Exp, scale=0.125
                                )
                                nc.tensor.matmul(
                                    ovs[h][:, cs:], V[:, kb, h, :], et[:, cs:],
                                    start=(kb == 0), stop=(kb == nkb - 1),
                                )
                        for h in heads:
                            base = 64 * (h % 2)
                            ov = ovs[h]
                            # gather denominator row; write unnormalized AT
                            nc.vector.tensor_copy(
                                sums_of[h][qb * 32 : qb * 32 + 1, :],
                                ov[DK : DK + 1, :],
                            )
                            nc.vector.tensor_copy(
                                AT[base : base + 64, pair, qsl], ov[0:DK, :]
                            )
                    # normalize: batched reciprocal over 8 rows, then per
                    # (h, qb): scatter row to partition 0, K=1 broadcast
                    # matmul, in-place multiply on AT.
                    srec_a = s2p.tile([128, 512], f32, tag="srec_a", bufs=2, name=f"srec_a{pair}")
                    srec_b = s2p.tile([128, 512], f32, tag="srec_b", bufs=2, name=f"srec_b{pair}")
                    nc.vector.reciprocal(srec_a[:], sums_a[:])
                    nc.vector.reciprocal(srec_b[:], sums_b[:])
                    srec_of = {heads[0]: srec_a, heads[1]: srec_b}
                    for qb in range(QB):
                        qsl = slice(qb * 512, (qb + 1) * 512)
                        for h in heads:
                            base = 64 * (h % 2)
                            rsr = s2p.tile([1, 512], f32r, tag="rsr", bufs=2, name=f"rsr{h}{qb}")
                            nc.vector.tensor_copy(
                                rsr[0:1, :], srec_of[h][qb * 32 : qb * 32 + 1, :]
                            )
                            rb = ps2.tile([64, 512], f32, tag="rb", bufs=1, name=f"rb{h}{qb}")
                            nc.tensor.matmul(
                                rb, ones_r[0:1, :], rsr[0:1, :], start=True, stop=True
                            )
                            nc.vector.tensor_tensor(
                                AT[base : base + 64, pair, qsl],
                                AT[base : base + 64, pair, qsl].bitcast(f32),
                                rb[0:64, :],
                                MULT,
                            )

              # ---------------- stage 3: output projection ----------------
              with (
                    tc.tile_pool(name="stage3", bufs=1) as s3p,
                    tc.tile_pool(name="ps3", bufs=1, space="PSUM") as ps3,
              ):
                    wo = s3p.tile([128, 4, D], f32r, tag="wo")
                    nc.sync.dma_start(
                        wo,
                        wo_d[:, :].rearrange("(io ip) j -> ip io j", ip=128).bitcast(f32r),
                    )
                    for sc in range(16):
                        og = s3p.tile([128, D], f32, tag="og", bufs=2, name=f"og{sc}")
                        for jh in range(2):
                            po = ps3.tile(
                                [128, 512], f32, tag="po", bufs=4, name=f"po{sc}{jh}"
                            )
                            for io in range(4):
                                nc.tensor.matmul(
                                    po,
                                    AT[:, io, sc * 128 : (sc + 1) * 128],
                                    wo[:, io, jh * 512 : (jh + 1) * 512],
                                    start=(io == 0), stop=(io == 3),
                                )
                            nc.any.tensor_copy(og[:, jh * 512 : (jh + 1) * 512], po[:])
                        nc.sync.dma_start(out_d[sc * 128 : (sc + 1) * 128, :], og[:])

    nc.compile()
    _BUILD_CACHE["nc"] = nc
    return nc


def _host_inputs(x, WQ, WK, WV, WO):
    ki = np.arange(128, dtype=np.float32)[:, None]
    qj = np.arange(128, dtype=np.float32)[None, :]
    # stripe mask: within the partial 128-col stripe of diagonal chunk d,
    # allowed iff (qj - 128d) >= ki, i.e. local column >= ki.
    mask = np.where(qj >= ki, 0.0, NEG).astype(np.float32)
    onesb = np.ones((128, 128), dtype=np.float32)

    in_maps = []
    for b in range(B):
        xT = np.ascontiguousarray(x[b].T)
        for g in range(G):
            sl = slice(g * DG, (g + 1) * DG)
            in_maps.append(
                {
                    "xT": xT,
                    "wqT": np.ascontiguousarray(WQ[sl, :].T),
                    "wkT": np.ascontiguousarray(WK[sl, :].T),
                    "wvT": np.ascontiguousarray(WV[sl, :].T),
                    "woT": np.ascontiguousarray(WO[:, sl].T),
                    "mask": mask,
                    "onesb": onesb,
                }
            )
    return in_maps


def kernel(x, WQ, WK, WV, WO):
    from concourse.bass_utils import run_bass_kernel_spmd

    x = np.asarray(x, dtype=np.float32)
    WQ = np.asarray(WQ, dtype=np.float32)
    WK = np.asarray(WK, dtype=np.float32)
    WV = np.asarray(WV, dtype=np.float32)
    WO = np.asarray(WO, dtype=np.float32)

    nc = _build()
    in_maps = _host_inputs(x, WQ, WK, WV, WO)
    res = run_bass_kernel_spmd(
        nc,
        in_maps,
        core_ids=list(range(8)),
        trace=bool(os.environ.get("KERNEL_TRACE")),
    )
    kernel.last_results = res
    parts = [r["out"] for r in res.results]
    out = np.stack([parts[2 * b] + parts[2 * b + 1] for b in range(B)], axis=0)
    return out.astype(np.float32)


# revision 5
# speedup vs baseline: 1.0627x; 1.0627x over previous
"""Causal multi-head self-attention on 8 Trainium2 NeuronCores.

Sharding: core = (batch b, head-group g).  B=4 batches x 2 groups of 8 heads
= 8 cores.  Each core computes Q/K/V projections for its 8 heads, causal
attention, and a partial output projection (row-shard of WO); the host sums
the two partials per batch (the tensor-parallel all-reduce, done at gather).

Per-core device pipeline (all matmuls in float32r = full fp32 precision via
the PE's 2-pass mode, 1 cycle/row at N>=256):
  stage 1: QT[d',s], KT[d',s] (transposed) and V[s,d'] (natural) projections
           from host-pre-transposed xT and weight shards.
  stage 2: q-block outer loop; per (q-block, head-pair): scoresT[k,q] per
           128-wide k-chunk, causal mask add on the single partial 128-col
           stripe of diagonal chunks, exp over the allowed column range only
           (no max subtraction -- scores are O(5) so exp is safe in fp32),
           attn@V with a ones column appended to V so PSUM row 64
           accumulates the softmax denominator.  The kb loop is software
           pipelined (attnV lags scores by one chunk) so the in-order PE
           stream never stalls on the ACT exp.  Denominators gather into
           [128, 512] tiles (rows at pair*32) so reciprocals run 8 rows at
           a time; normalization is an in-place multiply on AT.
  stage 3: output projection for the finished q rows, interleaved with the
           next q-block's attention.
"""

import os
import numpy as np

B, S, D = 4, 2048, 1024
H_TOTAL, DK = 16, 64
G = 2          # head groups (cores per batch)
HG = 8         # heads per core
DG = 512       # head dims per core
CO = 8         # contraction chunks of 128 over D
SBLK = 4       # 512-wide s blocks
QB = 4         # 512-wide q blocks
NEG = -1e9

_BUILD_CACHE = {}


def _build():
    if "nc" in _BUILD_CACHE:
        return _BUILD_CACHE["nc"]

    import concourse.bacc as bacc
    import concourse.mybir as mybir
    import concourse.tile as tile

    f32 = mybir.dt.float32
    f32r = mybir.dt.float32r
    AF = mybir.ActivationFunctionType
    ADD = mybir.AluOpType.add
    MULT = mybir.AluOpType.mult

    nc = bacc.Bacc("TRN2", target_bir_lowering=False)
    xT_d = nc.dram_tensor("xT", [D, S], f32, kind="ExternalInput")
    wq_d = nc.dram_tensor("wqT", [D, DG], f32, kind="ExternalInput")
    wk_d = nc.dram_tensor("wkT", [D, DG], f32, kind="ExternalInput")
    wv_d = nc.dram_tensor("wvT", [D, DG], f32, kind="ExternalInput")
    wo_d = nc.dram_tensor("woT", [DG, D], f32, kind="ExternalInput")
    mask_d = nc.dram_tensor("mask", [128, 128], f32, kind="ExternalInput")
    ones_d = nc.dram_tensor("onesb", [128, 128], f32, kind="ExternalInput")
    out_d = nc.dram_tensor("out", [S, D], f32, kind="ExternalOutput")

    with tile.TileContext(nc) as tc:
        with tc.tile_pool(name="persist", bufs=1) as pp:
            QT = pp.tile([128, 4, S], f32r, tag="QT")
            KT = pp.tile([128, 4, S], f32r, tag="KT")
            V = pp.tile([128, 16, HG, DK + 1], f32r, tag="V")
            maskb = pp.tile([128, 128], f32, tag="maskb")
            onesb = pp.tile([128, 128], f32, tag="onesb")
            ones_r = pp.tile([1, 64], f32r, tag="ones_r")
            nc.sync.dma_start(maskb[:], mask_d[:, :])
            nc.sync.dma_start(onesb[:], ones_d[:, :])
            nc.sync.dma_start(ones_r[:], ones_d[0:1, 0:64].bitcast(f32r))
            # ones column of V (f32 -> f32r rounding copy)
            nc.vector.tensor_copy(
                V[:, :, :, DK : DK + 1],
                onesb[:, 0:128].rearrange("p (so h) -> p so h", so=16)[:, :, :, None],
            )

            # ---------------- stage 1: projections ----------------
            with (
                tc.tile_pool(name="stage1", bufs=1) as s1p,
                tc.tile_pool(name="ps1", bufs=1, space="PSUM") as ps1,
            ):
                wq = s1p.tile([128, CO, DG], f32r, tag="wq")
                wk = s1p.tile([128, CO, DG], f32r, tag="wk")
                wv = s1p.tile([128, CO, DG], f32r, tag="wv")
                nc.sync.dma_start(
                    wq, wq_d[:, :].rearrange("(co ci) d -> ci co d", ci=128).bitcast(f32r)
                )
                nc.sync.dma_start(
                    wk, wk_d[:, :].rearrange("(co ci) d -> ci co d", ci=128).bitcast(f32r)
                )
                nc.sync.dma_start(
                    wv, wv_d[:, :].rearrange("(co ci) d -> ci co d", ci=128).bitcast(f32r)
                )
                for sb in range(SBLK):
                    xt = s1p.tile([128, CO, 512], f32r, tag="xt", bufs=2)
                    nc.sync.dma_start(
                        xt,
                        xT_d[:, sb * 512 : (sb + 1) * 512]
                        .rearrange("(co ci) s -> ci co s", ci=128)
                        .bitcast(f32r),
                    )
                    ssl = slice(sb * 512, (sb + 1) * 512)
                    for do in range(4):
                        dsl = slice(do * 128, (do + 1) * 128)
                        pq = ps1.tile([128, 512], f32, tag="proj", bufs=4, name=f"pq{sb}{do}")
                        for co in range(CO):
                            nc.tensor.matmul(
                                pq, wq[:, co, dsl], xt[:, co, :],
                                start=(co == 0), stop=(co == CO - 1),
                            )
                        nc.any.tensor_copy(QT[:, do, ssl], pq[:])
                        pk = ps1.tile([128, 512], f32, tag="proj", bufs=4, name=f"pk{sb}{do}")
                        for co in range(CO):
                            nc.tensor.matmul(
                                pk, wk[:, co, dsl], xt[:, co, :],
                                start=(co == 0), stop=(co == CO - 1),
                            )
                        nc.any.tensor_copy(KT[:, do, ssl], pk[:])
                    for so in range(4):
                        pv = ps1.tile([128, 512], f32, tag="proj", bufs=4, name=f"pv{sb}{so}")
                        for co in range(CO):
                            nc.tensor.matmul(
                                pv, xt[:, co, so * 128 : (so + 1) * 128], wv[:, co, :],
                                start=(co == 0), stop=(co == CO - 1),
                            )
                        nc.any.tensor_copy(
                            V[:, sb * 4 + so, :, 0:DK],
                            pv[:].rearrange("p (h d) -> p h d", h=HG),
                        )

            # ---------------- stages 2+3: attention + output ----------------
            with (
                tc.tile_pool(name="atp", bufs=1) as atp,
                tc.tile_pool(name="stage2", bufs=1) as s2p,
                tc.tile_pool(name="ps2", bufs=1, space="PSUM") as ps2,
            ):
                AT = atp.tile([128, 4, S], f32r, tag="AT")
                wo = s2p.tile([128, 4, D], f32r, tag="wo")
                nc.sync.dma_start(
                    wo,
                    wo_d[:, :].rearrange("(io ip) j -> ip io j", ip=128).bitcast(f32r),
                )
                for qb in range(QB):
                    qsl = slice(qb * 512, (qb + 1) * 512)
                    nkb = 4 * qb + 4
                    # denominator gather tiles; rows at pair*32 (SBUF
                    # partition offsets must be 32-aligned).  memset to 1.0
                    # so the batched reciprocal sees no garbage lanes.
                    sums_a = s2p.tile([128, 512], f32, tag="sums_a", bufs=2, name=f"sums_a{qb}")
                    sums_b = s2p.tile([128, 512], f32, tag="sums_b", bufs=2, name=f"sums_b{qb}")
                    nc.gpsimd.memset(sums_a[:], 1.0)
                    nc.gpsimd.memset(sums_b[:], 1.0)
                    for pair in range(4):
                        heads = (2 * pair, 2 * pair + 1)
                        sums_of = {heads[0]: sums_a, heads[1]: sums_b}
                        ovs = {}
                        for h in heads:
                            ov = ps2.tile(
                                [DK + 1, 512], f32, tag="ovpo", bufs=3, name=f"ov{h}q{qb}"
                            )
                            ovs[h] = ov
                        # software-pipelined kb loop: attnV lags scores by 1
                        pend = None  # (kb, cs, {h: et})
                        for kb in range(nkb):
                            ksl = slice(kb * 128, (kb + 1) * 128)
                            d = kb - 4 * qb
                            cs = 128 * d if d > 0 else 0
                            ets = {}
                            for h in heads:
                                base = 64 * (h % 2)
                                psl = slice(base, base + 64)
                                sp = ps2.tile(
                                    [128, 512], f32, tag="score", bufs=5,
                                    name=f"sp{h}q{qb}k{kb}",
                                )
                                nc.tensor.matmul(
                                    sp, KT[psl, pair, ksl], QT[psl, pair, qsl],
                                    start=True, stop=True,
                                )
                                if d >= 0:
                                    # only the 128-col stripe [128d, 128d+128)
                                    # is partially masked
                                    nc.vector.tensor_tensor(
                                        sp[:, cs : cs + 128],
                                        sp[:, cs : cs + 128],
                                        maskb[:, :],
                                        ADD,
                                    )
                                et = s2p.tile(
                                    [128, 512], f32r, tag="et", bufs=8,
                                    name=f"et{h}q{qb}k{kb}",
                                )
                                nc.scalar.activation(
                                    et[:, cs:], sp[:, cs:], AF.Exp, scale=0.125
                                )
                                ets[h] = et
                            if pend is not None:
                                pkb, pcs, pets = pend
                                for h in heads:
                                    nc.tensor.matmul(
                                        ovs[h][:, pcs:], V[:, pkb, h, :], pets[h][:, pcs:],
                                        start=(pkb == 0), stop=False,
                                    )
                            pend = (kb, cs, ets)
                        pkb, pcs, pets = pend
                        for h in heads:
                            nc.tensor.matmul(
                                ovs[h][:, pcs:], V[:, pkb, h, :], pets[h][:, pcs:],
                                start=(pkb == 0), stop=True,
                            )
                        for h in heads:
                            base = 64 * (h % 2)
                            ov = ovs[h]
                            # gather denominator row; write unnormalized AT
                            nc.vector.tensor_copy(
                                sums_of[h][pair * 32 : pair * 32 + 1, :],
                                ov[DK : DK + 1, :],
                            )
                            nc.vector.tensor_copy(
                                AT[base : base + 64, pair, qsl], ov[0:DK, :]
                            )
                    # normalize all 8 heads of this q-block: batched
                    # reciprocal, then per head a K=1 broadcast matmul and an
                    # in-place multiply on AT.
                    srec_a = s2p.tile([128, 512], f32, tag="srec_a", bufs=2, name=f"srec_a{qb}")
                    srec_b = s2p.tile([128, 512], f32, tag="srec_b", bufs=2, name=f"srec_b{qb}")
                    nc.vector.reciprocal(srec_a[:], sums_a[:])
                    nc.vector.reciprocal(srec_b[:], sums_b[:])
                    for pair in range(4):
                        heads = (2 * pair, 2 * pair + 1)
                        srec_of = {heads[0]: srec_a, heads[1]: srec_b}
                        for h in heads:
                            base = 64 * (h % 2)
                            rsr = s2p.tile([1, 512], f32r, tag="rsr", bufs=2, name=f"rsr{h}{qb}")
                            nc.vector.tensor_copy(
                                rsr[0:1, :], srec_of[h][pair * 32 : pair * 32 + 1, :]
                            )
                            rb = ps2.tile([128, 512], f32, tag="score", bufs=5, name=f"rb{h}{qb}")
                            nc.tensor.matmul(
                                rb[0:64, :], ones_r[0:1, :], rsr[0:1, :],
                                start=True, stop=True,
                            )
                            nc.vector.tensor_tensor(
                                AT[base : base + 64, pair, qsl],
                                AT[base : base + 64, pair, qsl].bitcast(f32),
                                rb[0:64, :],
                                MULT,
                            )
                    # output projection for the 4 finished 128-row s-chunks
                    for sc in range(4 * qb, 4 * qb + 4):
                        og = s2p.tile([128, D], f32, tag="og", bufs=2, name=f"og{sc}")
                        for jh in range(2):
                            po = ps2.tile(
                                [128, 512], f32, tag="ovpo", bufs=3, name=f"po{sc}{jh}"
                            )
                            for io in range(4):
                                nc.tensor.matmul(
                                    po,
                                    AT[:, io, sc * 128 : (sc + 1) * 128],
                                    wo[:, io, jh * 512 : (jh + 1) * 512],
                                    start=(io == 0), stop=(io == 3),
                                )
                            nc.vector.tensor_copy(og[:, jh * 512 : (jh + 1) * 512], po[:])
                        nc.sync.dma_start(out_d[sc * 128 : (sc + 1) * 128, :], og[:])

    nc.compile()
    _BUILD_CACHE["nc"] = nc
    return nc


def _host_inputs(x, WQ, WK, WV, WO):
    ki = np.arange(128, dtype=np.float32)[:, None]
    qj = np.arange(128, dtype=np.float32)[None, :]
    # stripe mask: within the partial 128-col stripe of diagonal chunk d,
    # allowed iff (qj - 128d) >= ki, i.e. local column >= ki.
    mask = np.where(qj >= ki, 0.0, NEG).astype(np.float32)
    onesb = np.ones((128, 128), dtype=np.float32)

    in_maps = []
    for b in range(B):
        xT = np.ascontiguousarray(x[b].T)
        for g in range(G):
            sl = slice(g * DG, (g + 1) * DG)
            in_maps.append(
                {
                    "xT": xT,
                    "wqT": np.ascontiguousarray(WQ[sl, :].T),
                    "wkT": np.ascontiguousarray(WK[sl, :].T),
                    "wvT": np.ascontiguousarray(WV[sl, :].T),
                    "woT": np.ascontiguousarray(WO[:, sl].T),
                    "mask": mask,
                    "onesb": onesb,
                }
            )
    return in_maps


def kernel(x, WQ, WK, WV, WO):
    from concourse.bass_utils import run_bass_kernel_spmd

    x = np.asarray(x, dtype=np.float32)
    WQ = np.asarray(WQ, dtype=np.float32)
    WK = np.asarray(WK, dtype=np.float32)
    WV = np.asarray(WV, dtype=np.float32)
    WO = np.asarray(WO, dtype=np.float32)

    nc = _build()
    in_maps = _host_inputs(x, WQ, WK, WV, WO)
    res = run_bass_kernel_spmd(
        nc,
        in_maps,
        core_ids=list(range(8)),
        trace=bool(os.environ.get("KERNEL_TRACE")),
    )
    kernel.last_results = res
    parts = [r["out"] for r in res.results]
    out = np.stack([parts[2 * b] + parts[2 * b + 1] for b in range(B)], axis=0)
    return out.astype(np.float32)


# revision 6
# speedup vs baseline: 1.0838x; 1.0198x over previous
"""Causal multi-head self-attention on 8 Trainium2 NeuronCores.

Sharding: core = (batch b, head-group g).  B=4 batches x 2 groups of 8 heads
= 8 cores.  Each core computes Q/K/V projections for its 8 heads, causal
attention, and a partial output projection (row-shard of WO); the host sums
the two partials per batch (the tensor-parallel all-reduce, done at gather).

Per-core device pipeline (all matmuls in float32r = full fp32 precision via
the PE's 2-pass mode, 1 cycle/row at N>=256):
  stage 1: QT[d',s], KT[d',s] (transposed) and V[s,d'] (natural) projections
           from host-pre-transposed xT and weight shards.
  stage 2: q-block outer loop; per (q-block, head-pair): scoresT[k,q] per
           128-wide k-chunk, causal mask add on the single partial 128-col
           stripe of diagonal chunks, exp over the allowed column range only
           (no max subtraction -- scores are O(5) so exp is safe in fp32),
           attn@V with a ones column appended to V so PSUM row 64
           accumulates the softmax denominator.  The kb loop is software
           pipelined (attnV lags scores by one chunk) so the in-order PE
           stream never stalls on the ACT exp.  Denominators gather into
           [128, 512] tiles (rows at pair*32) so reciprocals run 8 rows at
           a time; normalization is an in-place multiply on AT.
  stage 3: output projection for the finished q rows, interleaved with the
           next q-block's attention.
"""

import os
import numpy as np

B, S, D = 4, 2048, 1024
H_TOTAL, DK = 16, 64
G = 2          # head groups (cores per batch)
HG = 8         # heads per core
DG = 512       # head dims per core
CO = 8         # contraction chunks of 128 over D
SBLK = 4       # 512-wide s blocks
QB = 4         # 512-wide q blocks
NEG = -1e9

_BUILD_CACHE = {}


def _build():
    if "nc" in _BUILD_CACHE:
        return _BUILD_CACHE["nc"]

    import concourse.bacc as bacc
    import concourse.mybir as mybir
    import concourse.tile as tile

    f32 = mybir.dt.float32
    f32r = mybir.dt.float32r
    AF = mybir.ActivationFunctionType
    ADD = mybir.AluOpType.add
    MULT = mybir.AluOpType.mult

    nc = bacc.Bacc("TRN2", target_bir_lowering=False)
    xT_d = nc.dram_tensor("xT", [D, S], f32, kind="ExternalInput")
    wq_d = nc.dram_tensor("wqT", [D, DG], f32, kind="ExternalInput")
    wk_d = nc.dram_tensor("wkT", [D, DG], f32, kind="ExternalInput")
    wv_d = nc.dram_tensor("wvT", [D, DG], f32, kind="ExternalInput")
    wo_d = nc.dram_tensor("woT", [DG, D], f32, kind="ExternalInput")
    mask_d = nc.dram_tensor("mask", [128, 128], f32, kind="ExternalInput")
    ones_d = nc.dram_tensor("onesb", [128, 128], f32, kind="ExternalInput")
    out_d = nc.dram_tensor("out", [S, D], f32, kind="ExternalOutput")

    with tile.TileContext(nc) as tc:
        with tc.tile_pool(name="persist", bufs=1) as pp:
            QT = pp.tile([128, 4, S], f32r, tag="QT")
            KT = pp.tile([128, 4, S], f32r, tag="KT")
            V = pp.tile([128, 16, HG, DK + 1], f32r, tag="V")
            maskb = pp.tile([128, 128], f32, tag="maskb")
            onesb = pp.tile([128, 128], f32, tag="onesb")
            ones_r = pp.tile([1, 64], f32r, tag="ones_r")
            nc.sync.dma_start(maskb[:], mask_d[:, :])
            nc.sync.dma_start(onesb[:], ones_d[:, :])
            nc.sync.dma_start(ones_r[:], ones_d[0:1, 0:64].bitcast(f32r))
            # ones column of V (f32 -> f32r rounding copy)
            nc.vector.tensor_copy(
                V[:, :, :, DK : DK + 1],
                onesb[:, 0:128].rearrange("p (so h) -> p so h", so=16)[:, :, :, None],
            )

            # ---------------- stage 1: projections ----------------
            with (
                tc.tile_pool(name="stage1", bufs=1) as s1p,
                tc.tile_pool(name="ps1", bufs=1, space="PSUM") as ps1,
            ):
                wq = s1p.tile([128, CO, DG], f32r, tag="wq")
                wk = s1p.tile([128, CO, DG], f32r, tag="wk")
                wv = s1p.tile([128, CO, DG], f32r, tag="wv")
                nc.sync.dma_start(
                    wq, wq_d[:, :].rearrange("(co ci) d -> ci co d", ci=128).bitcast(f32r)
                )
                nc.sync.dma_start(
                    wk, wk_d[:, :].rearrange("(co ci) d -> ci co d", ci=128).bitcast(f32r)
                )
                nc.sync.dma_start(
                    wv, wv_d[:, :].rearrange("(co ci) d -> ci co d", ci=128).bitcast(f32r)
                )
                for sb in range(SBLK):
                    xt = s1p.tile([128, CO, 512], f32r, tag="xt", bufs=2)
                    nc.sync.dma_start(
                        xt,
                        xT_d[:, sb * 512 : (sb + 1) * 512]
                        .rearrange("(co ci) s -> ci co s", ci=128)
                        .bitcast(f32r),
                    )
                    ssl = slice(sb * 512, (sb + 1) * 512)
                    for do in range(4):
                        dsl = slice(do * 128, (do + 1) * 128)
                        pq = ps1.tile([128, 512], f32, tag="proj", bufs=4, name=f"pq{sb}{do}")
                        for co in range(CO):
                            nc.tensor.matmul(
                                pq, wq[:, co, dsl], xt[:, co, :],
                                start=(co == 0), stop=(co == CO - 1),
                            )
                        nc.any.tensor_copy(QT[:, do, ssl], pq[:])
                        pk = ps1.tile([128, 512], f32, tag="proj", bufs=4, name=f"pk{sb}{do}")
                        for co in range(CO):
                            nc.tensor.matmul(
                                pk, wk[:, co, dsl], xt[:, co, :],
                                start=(co == 0), stop=(co == CO - 1),
                            )
                        nc.any.tensor_copy(KT[:, do, ssl], pk[:])
                    for so in range(4):
                        pv = ps1.tile([128, 512], f32, tag="proj", bufs=4, name=f"pv{sb}{so}")
                        for co in range(CO):
                            nc.tensor.matmul(
                                pv, xt[:, co, so * 128 : (so + 1) * 128], wv[:, co, :],
                                start=(co == 0), stop=(co == CO - 1),
                            )
                        nc.any.tensor_copy(
                            V[:, sb * 4 + so, :, 0:DK],
                            pv[:].rearrange("p (h d) -> p h d", h=HG),
                        )

            # ---------------- stages 2+3: attention + output ----------------
            with (
                tc.tile_pool(name="atp", bufs=1) as atp,
                tc.tile_pool(name="stage2", bufs=1) as s2p,
                tc.tile_pool(name="ps2", bufs=1, space="PSUM") as ps2,
            ):
                AT = atp.tile([128, 4, S], f32r, tag="AT")
                wo = s2p.tile([128, 4, D], f32r, tag="wo")
                nc.sync.dma_start(
                    wo,
                    wo_d[:, :].rearrange("(io ip) j -> ip io j", ip=128).bitcast(f32r),
                )
                for qb in range(QB):
                    qsl = slice(qb * 512, (qb + 1) * 512)
                    nkb = 4 * qb + 4
                    # denominator gather tiles; rows at pair*32 (SBUF
                    # partition offsets must be 32-aligned).  memset to 1.0
                    # so the batched reciprocal sees no garbage lanes.
                    sums_a = s2p.tile([128, 512], f32, tag="sums_a", bufs=2, name=f"sums_a{qb}")
                    sums_b = s2p.tile([128, 512], f32, tag="sums_b", bufs=2, name=f"sums_b{qb}")
                    nc.gpsimd.memset(sums_a[:], 1.0)
                    nc.gpsimd.memset(sums_b[:], 1.0)
                    for pair in range(4):
                        heads = (2 * pair, 2 * pair + 1)
                        sums_of = {heads[0]: sums_a, heads[1]: sums_b}
                        ovs = {}
                        for h in heads:
                            ov = ps2.tile(
                                [DK + 1, 512], f32, tag="ovpo", bufs=2, name=f"ov{h}q{qb}"
                            )
                            ovs[h] = ov
                        # software-pipelined kb loop: attnV lags scores by 2
                        pend = []  # [(kb, cs, {h: et}), ...]
                        for kb in range(nkb):
                            ksl = slice(kb * 128, (kb + 1) * 128)
                            d = kb - 4 * qb
                            cs = 128 * d if d > 0 else 0
                            ets = {}
                            for h in heads:
                                base = 64 * (h % 2)
                                psl = slice(base, base + 64)
                                sp = ps2.tile(
                                    [128, 512], f32, tag="score", bufs=6,
                                    name=f"sp{h}q{qb}k{kb}",
                                )
                                nc.tensor.matmul(
                                    sp, KT[psl, pair, ksl], QT[psl, pair, qsl],
                                    start=True, stop=True,
                                )
                                if d >= 0:
                                    # only the 128-col stripe [128d, 128d+128)
                                    # is partially masked
                                    nc.vector.tensor_tensor(
                                        sp[:, cs : cs + 128],
                                        sp[:, cs : cs + 128],
                                        maskb[:, :],
                                        ADD,
                                    )
                                et = s2p.tile(
                                    [128, 512], f32r, tag="et", bufs=8,
                                    name=f"et{h}q{qb}k{kb}",
                                )
                                nc.scalar.activation(
                                    et[:, cs:], sp[:, cs:], AF.Exp, scale=0.125
                                )
                                ets[h] = et
                            pend.append((kb, cs, ets))
                            if len(pend) > 2:
                                pkb, pcs, pets = pend.pop(0)
                                for h in heads:
                                    nc.tensor.matmul(
                                        ovs[h][:, pcs:], V[:, pkb, h, :], pets[h][:, pcs:],
                                        start=(pkb == 0), stop=False,
                                    )
                        for pkb, pcs, pets in pend:
                            for h in heads:
                                nc.tensor.matmul(
                                    ovs[h][:, pcs:], V[:, pkb, h, :], pets[h][:, pcs:],
                                    start=(pkb == 0), stop=(pkb == nkb - 1),
                                )
                        for h in heads:
                            base = 64 * (h % 2)
                            ov = ovs[h]
                            # gather denominator row; write unnormalized AT
                            nc.vector.tensor_copy(
                                sums_of[h][pair * 32 : pair * 32 + 1, :],
                                ov[DK : DK + 1, :],
                            )
                            nc.vector.tensor_copy(
                                AT[base : base + 64, pair, qsl], ov[0:DK, :]
                            )
                    # normalize all 8 heads of this q-block: batched
                    # reciprocal, then per head a K=1 broadcast matmul and an
                    # in-place multiply on AT.
                    srec_a = s2p.tile([128, 512], f32, tag="srec_a", bufs=2, name=f"srec_a{qb}")
                    srec_b = s2p.tile([128, 512], f32, tag="srec_b", bufs=2, name=f"srec_b{qb}")
                    nc.vector.reciprocal(srec_a[:], sums_a[:])
                    nc.vector.reciprocal(srec_b[:], sums_b[:])
                    rsrs, rbs_ = {}, {}
                    for pair in range(4):
                        heads = (2 * pair, 2 * pair + 1)
                        srec_of = {heads[0]: srec_a, heads[1]: srec_b}
                        for h in heads:
                            rsr = s2p.tile([1, 512], f32r, tag="rsr", bufs=8, name=f"rsr{h}{qb}")
                            nc.vector.tensor_copy(
                                rsr[0:1, :], srec_of[h][pair * 32 : pair * 32 + 1, :]
                            )
                            rsrs[h] = rsr
                    for pair in range(4):
                        for h in (2 * pair, 2 * pair + 1):
                            rb = ps2.tile([128, 512], f32, tag="score", bufs=6, name=f"rb{h}{qb}")
                            nc.tensor.matmul(
                                rb[0:64, :], ones_r[0:1, :], rsrs[h][0:1, :],
                                start=True, stop=True,
                            )
                            rbs_[h] = rb
                    for pair in range(4):
                        for h in (2 * pair, 2 * pair + 1):
                            base = 64 * (h % 2)
                            nc.vector.tensor_tensor(
                                AT[base : base + 64, pair, qsl],
                                AT[base : base + 64, pair, qsl].bitcast(f32),
                                rbs_[h][0:64, :],
                                MULT,
                            )
                    # output projection for the 4 finished 128-row s-chunks
                    for sc in range(4 * qb, 4 * qb + 4):
                        og = s2p.tile([128, D], f32, tag="og", bufs=2, name=f"og{sc}")
                        for jh in range(2):
                            po = ps2.tile(
                                [128, 512], f32, tag="ovpo", bufs=2, name=f"po{sc}{jh}"
                            )
                            for io in range(4):
                                nc.tensor.matmul(
                                    po,
                                    AT[:, io, sc * 128 : (sc + 1) * 128],
                                    wo[:, io, jh * 512 : (jh + 1) * 512],
                                    start=(io == 0), stop=(io == 3),
                                )
                            nc.vector.tensor_copy(og[:, jh * 512 : (jh + 1) * 512], po[:])
                        nc.sync.dma_start(out_d[sc * 128 : (sc + 1) * 128, :], og[:])

    nc.compile()
    _BUILD_CACHE["nc"] = nc
    return nc


def _host_inputs(x, WQ, WK, WV, WO):
    ki = np.arange(128, dtype=np.float32)[:, None]
    qj = np.arange(128, dtype=np.float32)[None, :]
    # stripe mask: within the partial 128-col stripe of diagonal chunk d,
    # allowed iff (qj - 128d) >= ki, i.e. local column >= ki.
    mask = np.where(qj >= ki, 0.0, NEG).astype(np.float32)
    onesb = np.ones((128, 128), dtype=np.float32)

    in_maps = []
    for b in range(B):
        xT = np.ascontiguousarray(x[b].T)
        for g in range(G):
            sl = slice(g * DG, (g + 1) * DG)
            in_maps.append(
                {
                    "xT": xT,
                    "wqT": np.ascontiguousarray(WQ[sl, :].T),
                    "wkT": np.ascontiguousarray(WK[sl, :].T),
                    "wvT": np.ascontiguousarray(WV[sl, :].T),
                    "woT": np.ascontiguousarray(WO[:, sl].T),
                    "mask": mask,
                    "onesb": onesb,
                }
            )
    return in_maps


def kernel(x, WQ, WK, WV, WO):
    from concourse.bass_utils import run_bass_kernel_spmd

    x = np.asarray(x, dtype=np.float32)
    WQ = np.asarray(WQ, dtype=np.float32)
    WK = np.asarray(WK, dtype=np.float32)
    WV = np.asarray(WV, dtype=np.float32)
    WO = np.asarray(WO, dtype=np.float32)

    nc = _build()
    in_maps = _host_inputs(x, WQ, WK, WV, WO)
    res = run_bass_kernel_spmd(
        nc,
        in_maps,
        core_ids=list(range(8)),
        trace=bool(os.environ.get("KERNEL_TRACE")),
    )
    kernel.last_results = res
    parts = [r["out"] for r in res.results]
    out = np.stack([parts[2 * b] + parts[2 * b + 1] for b in range(B)], axis=0)
    return out.astype(np.float32)


# revision 7
# speedup vs baseline: 1.1463x; 1.0576x over previous
"""Causal multi-head self-attention on 8 Trainium2 NeuronCores.

Sharding: core = (batch b, head-group g).  B=4 batches x 2 groups of 8 heads
= 8 cores.  Each core computes Q/K/V projections for its 8 heads, causal
attention, and a partial output projection (row-shard of WO); the host sums
the two partials per batch (the tensor-parallel all-reduce, done at gather).

Per-core device pipeline (all matmuls in float32r = full fp32 precision via
the PE's 2-pass mode, 1 cycle/row at N>=256):
  stage 1: QT[d',s], KT[d',s] (transposed) and V[s,d'] (natural) projections
           from host-pre-transposed xT and weight shards.
  stage 2: q-block outer loop; per (q-block, head-pair): scoresT[k,q] per
           128-wide k-chunk, causal mask add on the single partial 128-col
           stripe of diagonal chunks, exp over the allowed column range only
           (no max subtraction -- scores are O(5) so exp is safe in fp32),
           attn@V with a ones column appended to V so PSUM row 64
           accumulates the softmax denominator.  The kb loop is software
           pipelined (attnV lags scores by one chunk) so the in-order PE
           stream never stalls on the ACT exp.  Denominators gather into
           [128, 512] tiles (rows at pair*32) so reciprocals run 8 rows at
           a time; normalization is an in-place multiply on AT.
  stage 3: output projection for the finished q rows, interleaved with the
           next q-block's attention.
"""

import os
import numpy as np

B, S, D = 4, 2048, 1024
H_TOTAL, DK = 16, 64
G = 2          # head groups (cores per batch)
HG = 8         # heads per core
DG = 512       # head dims per core
CO = 8         # contraction chunks of 128 over D
SBLK = 4       # 512-wide s blocks
QB = 4         # 512-wide q blocks
NEG = -1e9

_BUILD_CACHE = {}


def _build():
    if "nc" in _BUILD_CACHE:
        return _BUILD_CACHE["nc"]

    import concourse.bacc as bacc
    import concourse.mybir as mybir
    import concourse.tile as tile

    f32 = mybir.dt.float32
    f32r = mybir.dt.float32r
    AF = mybir.ActivationFunctionType
    ADD = mybir.AluOpType.add
    MULT = mybir.AluOpType.mult

    nc = bacc.Bacc("TRN2", target_bir_lowering=False)
    xT_d = nc.dram_tensor("xT", [D, S], f32, kind="ExternalInput")
    wq_d = nc.dram_tensor("wqT", [D, DG], f32, kind="ExternalInput")
    wk_d = nc.dram_tensor("wkT", [D, DG], f32, kind="ExternalInput")
    wv_d = nc.dram_tensor("wvT", [D, DG], f32, kind="ExternalInput")
    wo_d = nc.dram_tensor("woT", [DG, D], f32, kind="ExternalInput")
    mask_d = nc.dram_tensor("mask", [128, 128], f32, kind="ExternalInput")
    ones_d = nc.dram_tensor("onesb", [128, 128], f32, kind="ExternalInput")
    out_d = nc.dram_tensor("out", [S, D], f32, kind="ExternalOutput")

    with tile.TileContext(nc) as tc:
        with tc.tile_pool(name="persist", bufs=1) as pp:
            QT = pp.tile([128, 4, S], f32r, tag="QT")
            KT = pp.tile([128, 4, S], f32r, tag="KT")
            V = pp.tile([128, 16, HG, DK + 1], f32r, tag="V")
            maskb = pp.tile([128, 128], f32, tag="maskb")
            onesb = pp.tile([128, 128], f32, tag="onesb")
            ones_r = pp.tile([1, 64], f32r, tag="ones_r")
            nc.sync.dma_start(maskb[:], mask_d[:, :])
            nc.sync.dma_start(onesb[:], ones_d[:, :])
            nc.sync.dma_start(ones_r[:], ones_d[0:1, 0:64].bitcast(f32r))
            # ones column of V (f32 -> f32r rounding copy)
            nc.vector.tensor_copy(
                V[:, :, :, DK : DK + 1],
                onesb[:, 0:128].rearrange("p (so h) -> p so h", so=16)[:, :, :, None],
            )

            # ---------------- stage 1: projections ----------------
            with (
                tc.tile_pool(name="stage1", bufs=1) as s1p,
                tc.tile_pool(name="ps1", bufs=1, space="PSUM") as ps1,
            ):
                wq = s1p.tile([128, CO, DG], f32r, tag="wq")
                wk = s1p.tile([128, CO, DG], f32r, tag="wk")
                wv = s1p.tile([128, CO, DG], f32r, tag="wv")
                nc.sync.dma_start(
                    wq, wq_d[:, :].rearrange("(co ci) d -> ci co d", ci=128).bitcast(f32r)
                )
                nc.sync.dma_start(
                    wk, wk_d[:, :].rearrange("(co ci) d -> ci co d", ci=128).bitcast(f32r)
                )
                nc.sync.dma_start(
                    wv, wv_d[:, :].rearrange("(co ci) d -> ci co d", ci=128).bitcast(f32r)
                )
                for sb in range(SBLK):
                    xt = s1p.tile([128, CO, 512], f32r, tag="xt", bufs=2)
                    nc.sync.dma_start(
                        xt,
                        xT_d[:, sb * 512 : (sb + 1) * 512]
                        .rearrange("(co ci) s -> ci co s", ci=128)
                        .bitcast(f32r),
                    )
                    ssl = slice(sb * 512, (sb + 1) * 512)
                    for do in range(4):
                        dsl = slice(do * 128, (do + 1) * 128)
                        pq = ps1.tile([128, 512], f32, tag="proj", bufs=4, name=f"pq{sb}{do}")
                        for co in range(CO):
                            nc.tensor.matmul(
                                pq, wq[:, co, dsl], xt[:, co, :],
                                start=(co == 0), stop=(co == CO - 1),
                            )
                        nc.any.tensor_copy(QT[:, do, ssl], pq[:])
                        pk = ps1.tile([128, 512], f32, tag="proj", bufs=4, name=f"pk{sb}{do}")
                        for co in range(CO):
                            nc.tensor.matmul(
                                pk, wk[:, co, dsl], xt[:, co, :],
                                start=(co == 0), stop=(co == CO - 1),
                            )
                        nc.any.tensor_copy(KT[:, do, ssl], pk[:])
                    for so in range(4):
                        pv = ps1.tile([128, 512], f32, tag="proj", bufs=4, name=f"pv{sb}{so}")
                        for co in range(CO):
                            nc.tensor.matmul(
                                pv, xt[:, co, so * 128 : (so + 1) * 128], wv[:, co, :],
                                start=(co == 0), stop=(co == CO - 1),
                            )
                        nc.any.tensor_copy(
                            V[:, sb * 4 + so, :, 0:DK],
                            pv[:].rearrange("p (h d) -> p h d", h=HG),
                        )

            # ---------------- stages 2+3: attention + output ----------------
            with (
                tc.tile_pool(name="atp", bufs=1) as atp,
                tc.tile_pool(name="stage2", bufs=1) as s2p,
                tc.tile_pool(name="ps2", bufs=1, space="PSUM") as ps2,
            ):
                AT = atp.tile([128, 4, S], f32r, tag="AT")
                wo = s2p.tile([128, 4, D], f32r, tag="wo")
                nc.sync.dma_start(
                    wo,
                    wo_d[:, :].rearrange("(io ip) j -> ip io j", ip=128).bitcast(f32r),
                )
                for qb in range(QB):
                    qsl = slice(qb * 512, (qb + 1) * 512)
                    nkb = 4 * qb + 4
                    # denominator gather tiles; rows at pair*32 (SBUF
                    # partition offsets must be 32-aligned).  memset to 1.0
                    # so the batched reciprocal sees no garbage lanes.
                    sums_a = s2p.tile([128, 512], f32, tag="sums_a", bufs=1, name=f"sums_a{qb}")
                    sums_b = s2p.tile([128, 512], f32, tag="sums_b", bufs=1, name=f"sums_b{qb}")
                    nc.gpsimd.memset(sums_a[:], 1.0)
                    nc.gpsimd.memset(sums_b[:], 1.0)
                    for pair in range(4):
                        heads = (2 * pair, 2 * pair + 1)
                        sums_of = {heads[0]: sums_a, heads[1]: sums_b}
                        ovs = {}
                        for h in heads:
                            ov = ps2.tile(
                                [DK + 1, 512], f32, tag="ovpo", bufs=2, name=f"ov{h}q{qb}"
                            )
                            ovs[h] = ov
                        # chunked kb loop with one-chunk lag: emit a
                        # chunk of scores+exps, then the PREVIOUS chunk's
                        # attnV matmuls grouped per head (consecutive
                        # same-bank accumulation -- interleaving score and
                        # attnV matmuls forces a PE pipeline drain at every
                        # lhsT row-group conflict, measured 733 vs 237 ns/mm)
                        def emit_avs(items):
                            for h in heads:
                                for (pkb, pcs, pets) in items:
                                    nc.tensor.matmul(
                                        ovs[h][:, pcs:], V[:, pkb, h, :],
                                        pets[h][:, pcs:],
                                        start=(pkb == 0), stop=(pkb == nkb - 1),
                                    )

                        CH = 3
                        kbs = list(range(nkb))
                        chunks = [kbs[i : i + CH] for i in range(0, nkb, CH)]
                        pend = None
                        for chunk in chunks:
                            items = []
                            for kb in chunk:
                                ksl = slice(kb * 128, (kb + 1) * 128)
                                d = kb - 4 * qb
                                cs = 128 * d if d > 0 else 0
                                ets = {}
                                for h in heads:
                                    base = 64 * (h % 2)
                                    psl = slice(base, base + 64)
                                    sp = ps2.tile(
                                        [128, 512], f32, tag="score", bufs=6,
                                        name=f"sp{h}q{qb}k{kb}",
                                    )
                                    nc.tensor.matmul(
                                        sp, KT[psl, pair, ksl], QT[psl, pair, qsl],
                                        start=True, stop=True,
                                    )
                                    if d >= 0:
                                        nc.vector.tensor_tensor(
                                            sp[:, cs : cs + 128],
                                            sp[:, cs : cs + 128],
                                            maskb[:, :],
                                            ADD,
                                        )
                                    et = s2p.tile(
                                        [128, 512], f32r, tag="et", bufs=12,
                                        name=f"et{h}q{qb}k{kb}",
                                    )
                                    nc.scalar.activation(
                                        et[:, cs:], sp[:, cs:], AF.Exp, scale=0.125
                                    )
                                    ets[h] = et
                                items.append((kb, cs, ets))
                            if pend is not None:
                                emit_avs(pend)
                            pend = items
                        emit_avs(pend)
                        for h in heads:
                            base = 64 * (h % 2)
                            ov = ovs[h]
                            # gather denominator row; write unnormalized AT
                            nc.vector.tensor_copy(
                                sums_of[h][pair * 32 : pair * 32 + 1, :],
                                ov[DK : DK + 1, :],
                            )
                            nc.vector.tensor_copy(
                                AT[base : base + 64, pair, qsl], ov[0:DK, :]
                            )
                    # normalize all 8 heads of this q-block: batched
                    # reciprocal, then per head a K=1 broadcast matmul and an
                    # in-place multiply on AT.
                    srec_a = s2p.tile([128, 512], f32, tag="srec_a", bufs=1, name=f"srec_a{qb}")
                    srec_b = s2p.tile([128, 512], f32, tag="srec_b", bufs=1, name=f"srec_b{qb}")
                    nc.vector.reciprocal(srec_a[:], sums_a[:])
                    nc.vector.reciprocal(srec_b[:], sums_b[:])
                    rsrs, rbs_ = {}, {}
                    for pair in range(4):
                        heads = (2 * pair, 2 * pair + 1)
                        srec_of = {heads[0]: srec_a, heads[1]: srec_b}
                        for h in heads:
                            rsr = s2p.tile([1, 512], f32r, tag="rsr", bufs=8, name=f"rsr{h}{qb}")
                            nc.vector.tensor_copy(
                                rsr[0:1, :], srec_of[h][pair * 32 : pair * 32 + 1, :]
                            )
                            rsrs[h] = rsr
                    for pair in range(4):
                        for h in (2 * pair, 2 * pair + 1):
                            rb = ps2.tile([128, 512], f32, tag="score", bufs=6, name=f"rb{h}{qb}")
                            nc.tensor.matmul(
                                rb[0:64, :], ones_r[0:1, :], rsrs[h][0:1, :],
                                start=True, stop=True,
                            )
                            rbs_[h] = rb
                    for pair in range(4):
                        for h in (2 * pair, 2 * pair + 1):
                            base = 64 * (h % 2)
                            nc.vector.tensor_tensor(
                                AT[base : base + 64, pair, qsl],
                                AT[base : base + 64, pair, qsl].bitcast(f32),
                                rbs_[h][0:64, :],
                                MULT,
                            )
                    # output projection for the 4 finished 128-row s-chunks
                    for sc in range(4 * qb, 4 * qb + 4):
                        og = s2p.tile([128, D], f32, tag="og", bufs=2, name=f"og{sc}")
                        for jh in range(2):
                            po = ps2.tile(
                                [128, 512], f32, tag="ovpo", bufs=2, name=f"po{sc}{jh}"
                            )
                            for io in range(4):
                                nc.tensor.matmul(
                                    po,
                                    AT[:, io, sc * 128 : (sc + 1) * 128],
                                    wo[:, io, jh * 512 : (jh + 1) * 512],
                                    start=(io == 0), stop=(io == 3),
                                )
                            nc.vector.tensor_copy(og[:, jh * 512 : (jh + 1) * 512], po[:])
                        nc.sync.dma_start(out_d[sc * 128 : (sc + 1) * 128, :], og[:])

    nc.compile()
    _BUILD_CACHE["nc"] = nc
    return nc


def _host_inputs(x, WQ, WK, WV, WO):
    ki = np.arange(128, dtype=np.float32)[:, None]
    qj = np.arange(128, dtype=np.float32)[None, :]
    # stripe mask: within the partial 128-col stripe of diagonal chunk d,
    # allowed iff (qj - 128d) >= ki, i.e. local column >= ki.
    mask = np.where(qj >= ki, 0.0, NEG).astype(np.float32)
    onesb = np.ones((128, 128), dtype=np.float32)

    in_maps = []
    for b in range(B):
        xT = np.ascontiguousarray(x[b].T)
        for g in range(G):
            sl = slice(g * DG, (g + 1) * DG)
            in_maps.append(
                {
                    "xT": xT,
                    "wqT": np.ascontiguousarray(WQ[sl, :].T),
                    "wkT": np.ascontiguousarray(WK[sl, :].T),
                    "wvT": np.ascontiguousarray(WV[sl, :].T),
                    "woT": np.ascontiguousarray(WO[:, sl].T),
                    "mask": mask,
                    "onesb": onesb,
                }
            )
    return in_maps


def kernel(x, WQ, WK, WV, WO):
    from concourse.bass_utils import run_bass_kernel_spmd

    x = np.asarray(x, dtype=np.float32)
    WQ = np.asarray(WQ, dtype=np.float32)
    WK = np.asarray(WK, dtype=np.float32)
    WV = np.asarray(WV, dtype=np.float32)
    WO = np.asarray(WO, dtype=np.float32)

    nc = _build()
    in_maps = _host_inputs(x, WQ, WK, WV, WO)
    res = run_bass_kernel_spmd(
        nc,
        in_maps,
        core_ids=list(range(8)),
        trace=bool(os.environ.get("KERNEL_TRACE")),
    )
    kernel.last_results = res
    parts = [r["out"] for r in res.results]
    out = np.stack([parts[2 * b] + parts[2 * b + 1] for b in range(B)], axis=0)
    return out.astype(np.float32)


# revision 8
# speedup vs baseline: 1.4059x; 1.2265x over previous
"""Causal multi-head self-attention on 8 Trainium2 NeuronCores.

Sharding: core = (batch b, head-group g).  B=4 batches x 2 groups of 8 heads
= 8 cores.  Each core computes Q/K/V projections for its 8 heads, causal
attention, and a partial output projection (row-shard of WO); the host sums
the two partials per batch (the tensor-parallel all-reduce, done at gather).

Per-core device pipeline (all matmuls in float32r = full fp32 precision via
the PE's 2-pass mode, 1 cycle/row at N>=256):
  stage 1: QT[d',s], KT[d',s] (transposed) and V[s,d'] (natural) projections
           from host-pre-transposed xT and weight shards.
  stage 2: q-block outer loop; per (q-block, head-pair): scoresT[k,q] per
           128-wide k-chunk, causal mask add on the single partial 128-col
           stripe of diagonal chunks, exp over the allowed column range only
           (no max subtraction -- scores are O(5) so exp is safe in fp32),
           attn@V with a ones column appended to V so PSUM row 64
           accumulates the softmax denominator.  The kb loop is software
           pipelined (attnV lags scores by one chunk) so the in-order PE
           stream never stalls on the ACT exp.  Denominators gather into
           [128, 512] tiles (rows at pair*32) so reciprocals run 8 rows at
           a time; normalization is an in-place multiply on AT.
  stage 3: output projection for the finished q rows, interleaved with the
           next q-block's attention.
"""

import os
import numpy as np

B, S, D = 4, 2048, 1024
H_TOTAL, DK = 16, 64
G = 2          # head groups (cores per batch)
HG = 8         # heads per core
DG = 512       # head dims per core
CO = 8         # contraction chunks of 128 over D
SBLK = 4       # 512-wide s blocks
QB = 4         # 512-wide q blocks
NEG = -1e9

_BUILD_CACHE = {}


def _build():
    if "nc" in _BUILD_CACHE:
        return _BUILD_CACHE["nc"]

    import concourse.bacc as bacc
    import concourse.mybir as mybir
    import concourse.tile as tile
    from concourse.tile_rust import add_dep_helper

    f32 = mybir.dt.float32
    f32r = mybir.dt.float32r
    AF = mybir.ActivationFunctionType
    ADD = mybir.AluOpType.add
    MULT = mybir.AluOpType.mult

    nc = bacc.Bacc("TRN2", target_bir_lowering=False)
    xT_d = nc.dram_tensor("xT", [D, S], f32, kind="ExternalInput")
    wq_d = nc.dram_tensor("wqT", [D, DG], f32, kind="ExternalInput")
    wk_d = nc.dram_tensor("wkT", [D, DG], f32, kind="ExternalInput")
    wv_d = nc.dram_tensor("wvT", [D, DG], f32, kind="ExternalInput")
    wo_d = nc.dram_tensor("woT", [DG, D], f32, kind="ExternalInput")
    mask_d = nc.dram_tensor("mask", [128, 128], f32, kind="ExternalInput")
    ones_d = nc.dram_tensor("onesb", [128, 128], f32, kind="ExternalInput")
    out_d = nc.dram_tensor("out", [S, D], f32, kind="ExternalOutput")

    with tile.TileContext(nc) as tc:
        with tc.tile_pool(name="persist", bufs=1) as pp:
            QT = pp.tile([128, 4, S], f32r, tag="QT")
            KT = pp.tile([128, 4, S], f32r, tag="KT")
            V = pp.tile([128, 16, HG, DK + 1], f32r, tag="V")
            maskb = pp.tile([128, 128], f32, tag="maskb")
            onesb = pp.tile([128, 128], f32, tag="onesb")
            ones_r = pp.tile([1, 64], f32r, tag="ones_r")
            nc.sync.dma_start(maskb[:], mask_d[:, :])
            nc.sync.dma_start(onesb[:], ones_d[:, :])
            nc.sync.dma_start(ones_r[:], ones_d[0:1, 0:64].bitcast(f32r))
            # ones column of V (f32 -> f32r rounding copy)
            nc.vector.tensor_copy(
                V[:, :, :, DK : DK + 1],
                onesb[:, 0:128].rearrange("p (so h) -> p so h", so=16)[:, :, :, None],
            )

            # ---------------- stage 1: projections ----------------
            with (
                tc.tile_pool(name="stage1", bufs=1) as s1p,
                tc.tile_pool(name="ps1", bufs=1, space="PSUM") as ps1,
            ):
                wq = s1p.tile([128, CO, DG], f32r, tag="wq")
                wk = s1p.tile([128, CO, DG], f32r, tag="wk")
                wv = s1p.tile([128, CO, DG], f32r, tag="wv")
                nc.sync.dma_start(
                    wq, wq_d[:, :].rearrange("(co ci) d -> ci co d", ci=128).bitcast(f32r)
                )
                nc.sync.dma_start(
                    wk, wk_d[:, :].rearrange("(co ci) d -> ci co d", ci=128).bitcast(f32r)
                )
                nc.sync.dma_start(
                    wv, wv_d[:, :].rearrange("(co ci) d -> ci co d", ci=128).bitcast(f32r)
                )
                for sb in range(SBLK):
                    xt = s1p.tile([128, CO, 512], f32r, tag="xt", bufs=2)
                    nc.sync.dma_start(
                        xt,
                        xT_d[:, sb * 512 : (sb + 1) * 512]
                        .rearrange("(co ci) s -> ci co s", ci=128)
                        .bitcast(f32r),
                    )
                    ssl = slice(sb * 512, (sb + 1) * 512)
                    for do in range(4):
                        dsl = slice(do * 128, (do + 1) * 128)
                        pq = ps1.tile([128, 512], f32, tag="proj", bufs=4, name=f"pq{sb}{do}")
                        for co in range(CO):
                            nc.tensor.matmul(
                                pq, wq[:, co, dsl], xt[:, co, :],
                                start=(co == 0), stop=(co == CO - 1),
                            )
                        nc.any.tensor_copy(QT[:, do, ssl], pq[:])
                        pk = ps1.tile([128, 512], f32, tag="proj", bufs=4, name=f"pk{sb}{do}")
                        for co in range(CO):
                            nc.tensor.matmul(
                                pk, wk[:, co, dsl], xt[:, co, :],
                                start=(co == 0), stop=(co == CO - 1),
                            )
                        nc.any.tensor_copy(KT[:, do, ssl], pk[:])
                    for so in range(4):
                        pv = ps1.tile([128, 512], f32, tag="proj", bufs=4, name=f"pv{sb}{so}")
                        for co in range(CO):
                            nc.tensor.matmul(
                                pv, xt[:, co, so * 128 : (so + 1) * 128], wv[:, co, :],
                                start=(co == 0), stop=(co == CO - 1),
                            )
                        nc.any.tensor_copy(
                            V[:, sb * 4 + so, :, 0:DK],
                            pv[:].rearrange("p (h d) -> p h d", h=HG),
                        )

            # ---------------- stages 2+3: attention + output ----------------
            with (
                tc.tile_pool(name="atp", bufs=1) as atp,
                tc.tile_pool(name="stage2", bufs=1) as s2p,
                tc.tile_pool(name="ps2", bufs=1, space="PSUM") as ps2,
            ):
                AT = atp.tile([128, 4, S], f32r, tag="AT")
                wo = s2p.tile([128, 4, D], f32r, tag="wo")
                nc.sync.dma_start(
                    wo,
                    wo_d[:, :].rearrange("(io ip) j -> ip io j", ip=128).bitcast(f32r),
                )
                pe_prev = [None]  # last instr of the previous PE group

                def pe_group(insts):
                    # force PE issue order: first of this group after last of
                    # the previous group; chain within the group
                    if not insts:
                        return
                    if pe_prev[0] is not None:
                        add_dep_helper(
                            insts[0].ins, pe_prev[0].ins, sync=False,
                            reason="pe group order",
                        )
                    for a, b in zip(insts[1:], insts):
                        add_dep_helper(a.ins, b.ins, sync=False, reason="pe chain")
                    pe_prev[0] = insts[-1]

                for qb in range(QB):
                    qsl = slice(qb * 512, (qb + 1) * 512)
                    nkb = 4 * qb + 4
                    # denominator gather tiles; rows at pair*32 (SBUF
                    # partition offsets must be 32-aligned).  memset to 1.0
                    # so the batched reciprocal sees no garbage lanes.
                    sums_a = s2p.tile([128, 512], f32, tag="sums_a", bufs=1, name=f"sums_a{qb}")
                    sums_b = s2p.tile([128, 512], f32, tag="sums_b", bufs=1, name=f"sums_b{qb}")
                    nc.gpsimd.memset(sums_a[:], 1.0)
                    nc.gpsimd.memset(sums_b[:], 1.0)
                    for pair in range(4):
                        heads = (2 * pair, 2 * pair + 1)
                        sums_of = {heads[0]: sums_a, heads[1]: sums_b}
                        ovs = {}
                        for h in heads:
                            ov = ps2.tile(
                                [DK + 1, 512], f32, tag="ovpo", bufs=2, name=f"ov{h}q{qb}"
                            )
                            ovs[h] = ov
                        # chunked kb loop with one-chunk lag: emit a
                        # chunk of scores+exps, then the PREVIOUS chunk's
                        # attnV matmuls grouped per head (consecutive
                        # same-bank accumulation -- interleaving score and
                        # attnV matmuls forces a PE pipeline drain at every
                        # lhsT row-group conflict, measured 733 vs 237 ns/mm)
                        def emit_avs(items):
                            grp = []
                            for h in heads:
                                for (pkb, pcs, pets) in items:
                                    grp.append(nc.tensor.matmul(
                                        ovs[h][:, pcs:], V[:, pkb, h, :],
                                        pets[h][:, pcs:],
                                        start=(pkb == 0), stop=(pkb == nkb - 1),
                                    ))
                            pe_group(grp)

                        CH = 3
                        kbs = list(range(nkb))
                        chunks = [kbs[i : i + CH] for i in range(0, nkb, CH)]
                        pend = None
                        for chunk in chunks:
                            items = []
                            sc_grp = []
                            for kb in chunk:
                                ksl = slice(kb * 128, (kb + 1) * 128)
                                d = kb - 4 * qb
                                cs = 128 * d if d > 0 else 0
                                ets = {}
                                for h in heads:
                                    base = 64 * (h % 2)
                                    psl = slice(base, base + 64)
                                    sp = ps2.tile(
                                        [128, 512], f32, tag="score", bufs=6,
                                        name=f"sp{h}q{qb}k{kb}",
                                    )
                                    sc_grp.append(nc.tensor.matmul(
                                        sp, KT[psl, pair, ksl], QT[psl, pair, qsl],
                                        start=True, stop=True,
                                    ))
                                    if d >= 0:
                                        nc.vector.tensor_tensor(
                                            sp[:, cs : cs + 128],
                                            sp[:, cs : cs + 128],
                                            maskb[:, :],
                                            ADD,
                                        )
                                    et = s2p.tile(
                                        [128, 512], f32r, tag="et", bufs=12,
                                        name=f"et{h}q{qb}k{kb}",
                                    )
                                    nc.scalar.activation(
                                        et[:, cs:], sp[:, cs:], AF.Exp, scale=0.125
                                    )
                                    ets[h] = et
                                items.append((kb, cs, ets))
                            pe_group(sc_grp)
                            if pend is not None:
                                emit_avs(pend)
                            pend = items
                        emit_avs(pend)
                        for h in heads:
                            base = 64 * (h % 2)
                            ov = ovs[h]
                            # gather denominator row; write unnormalized AT
                            nc.vector.tensor_copy(
                                sums_of[h][pair * 32 : pair * 32 + 1, :],
                                ov[DK : DK + 1, :],
                            )
                            nc.vector.tensor_copy(
                                AT[base : base + 64, pair, qsl], ov[0:DK, :]
                            )
                    # normalize all 8 heads of this q-block: batched
                    # reciprocal, then per head a K=1 broadcast matmul and an
                    # in-place multiply on AT.
                    srec_a = s2p.tile([128, 512], f32, tag="srec_a", bufs=1, name=f"srec_a{qb}")
                    srec_b = s2p.tile([128, 512], f32, tag="srec_b", bufs=1, name=f"srec_b{qb}")
                    nc.vector.reciprocal(srec_a[:], sums_a[:])
                    nc.vector.reciprocal(srec_b[:], sums_b[:])
                    rsrs, rbs_ = {}, {}
                    for pair in range(4):
                        heads = (2 * pair, 2 * pair + 1)
                        srec_of = {heads[0]: srec_a, heads[1]: srec_b}
                        for h in heads:
                            rsr = s2p.tile([1, 512], f32r, tag="rsr", bufs=8, name=f"rsr{h}{qb}")
                            nc.vector.tensor_copy(
                                rsr[0:1, :], srec_of[h][pair * 32 : pair * 32 + 1, :]
                            )
                            rsrs[h] = rsr
                    rb_grp = []
                    for pair in range(4):
                        for h in (2 * pair, 2 * pair + 1):
                            rb = ps2.tile([128, 512], f32, tag="score", bufs=6, name=f"rb{h}{qb}")
                            rb_grp.append(nc.tensor.matmul(
                                rb[0:64, :], ones_r[0:1, :], rsrs[h][0:1, :],
                                start=True, stop=True,
                            ))
                            rbs_[h] = rb
                    pe_group(rb_grp)
                    for pair in range(4):
                        for h in (2 * pair, 2 * pair + 1):
                            base = 64 * (h % 2)
                            nc.vector.tensor_tensor(
                                AT[base : base + 64, pair, qsl],
                                AT[base : base + 64, pair, qsl].bitcast(f32),
                                rbs_[h][0:64, :],
                                MULT,
                            )
                    # output projection for the 4 finished 128-row s-chunks
                    for sc in range(4 * qb, 4 * qb + 4):
                        og = s2p.tile([128, D], f32, tag="og", bufs=2, name=f"og{sc}")
                        po_grp = []
                        for jh in range(2):
                            po = ps2.tile(
                                [128, 512], f32, tag="ovpo", bufs=2, name=f"po{sc}{jh}"
                            )
                            for io in range(4):
                                po_grp.append(nc.tensor.matmul(
                                    po,
                                    AT[:, io, sc * 128 : (sc + 1) * 128],
                                    wo[:, io, jh * 512 : (jh + 1) * 512],
                                    start=(io == 0), stop=(io == 3),
                                ))
                            nc.vector.tensor_copy(og[:, jh * 512 : (jh + 1) * 512], po[:])
                        pe_group(po_grp)
                        nc.sync.dma_start(out_d[sc * 128 : (sc + 1) * 128, :], og[:])

    nc.compile()
    _BUILD_CACHE["nc"] = nc
    return nc


def _host_inputs(x, WQ, WK, WV, WO):
    ki = np.arange(128, dtype=np.float32)[:, None]
    qj = np.arange(128, dtype=np.float32)[None, :]
    # stripe mask: within the partial 128-col stripe of diagonal chunk d,
    # allowed iff (qj - 128d) >= ki, i.e. local column >= ki.
    mask = np.where(qj >= ki, 0.0, NEG).astype(np.float32)
    onesb = np.ones((128, 128), dtype=np.float32)

    in_maps = []
    for b in range(B):
        xT = np.ascontiguousarray(x[b].T)
        for g in range(G):
            sl = slice(g * DG, (g + 1) * DG)
            in_maps.append(
                {
                    "xT": xT,
                    "wqT": np.ascontiguousarray(WQ[sl, :].T),
                    "wkT": np.ascontiguousarray(WK[sl, :].T),
                    "wvT": np.ascontiguousarray(WV[sl, :].T),
                    "woT": np.ascontiguousarray(WO[:, sl].T),
                    "mask": mask,
                    "onesb": onesb,
                }
            )
    return in_maps


def kernel(x, WQ, WK, WV, WO):
    from concourse.bass_utils import run_bass_kernel_spmd

    x = np.asarray(x, dtype=np.float32)
    WQ = np.asarray(WQ, dtype=np.float32)
    WK = np.asarray(WK, dtype=np.float32)
    WV = np.asarray(WV, dtype=np.float32)
    WO = np.asarray(WO, dtype=np.float32)

    nc = _build()
    in_maps = _host_inputs(x, WQ, WK, WV, WO)
    res = run_bass_kernel_spmd(
        nc,
        in_maps,
        core_ids=list(range(8)),
        trace=bool(os.environ.get("KERNEL_TRACE")),
    )
    kernel.last_results = res
    parts = [r["out"] for r in res.results]
    out = np.stack([parts[2 * b] + parts[2 * b + 1] for b in range(B)], axis=0)
    return out.astype(np.float32)


# revision 9
# speedup vs baseline: 1.4486x; 1.0304x over previous
"""Causal multi-head self-attention on 8 Trainium2 NeuronCores.

Sharding: core = (batch b, head-group g).  B=4 batches x 2 groups of 8 heads
= 8 cores.  Each core computes Q/K/V projections for its 8 heads, causal
attention, and a partial output projection (row-shard of WO); the host sums
the two partials per batch (the tensor-parallel all-reduce, done at gather).

Per-core device pipeline (all matmuls in float32r = full fp32 precision via
the PE's 2-pass mode, 1 cycle/row at N>=256):
  stage 1: QT[d',s], KT[d',s] (transposed) and V[s,d'] (natural) projections
           from host-pre-transposed xT and weight shards.
  stage 2: q-block outer loop; per (q-block, head-pair): scoresT[k,q] per
           128-wide k-chunk, causal mask add on the single partial 128-col
           stripe of diagonal chunks, exp over the allowed column range only
           (no max subtraction -- scores are O(5) so exp is safe in fp32),
           attn@V with a ones column appended to V so PSUM row 64
           accumulates the softmax denominator.  The kb loop is software
           pipelined (attnV lags scores by one chunk) so the in-order PE
           stream never stalls on the ACT exp.  Denominators gather into
           [128, 512] tiles (rows at pair*32) so reciprocals run 8 rows at
           a time; normalization is an in-place multiply on AT.
  stage 3: output projection for the finished q rows, interleaved with the
           next q-block's attention.
"""

import os
import numpy as np

B, S, D = 4, 2048, 1024
H_TOTAL, DK = 16, 64
G = 2          # head groups (cores per batch)
HG = 8         # heads per core
DG = 512       # head dims per core
CO = 8         # contraction chunks of 128 over D
SBLK = 4       # 512-wide s blocks
QB = 4         # 512-wide q blocks
NEG = -1e9

_BUILD_CACHE = {}


def _build():
    if "nc" in _BUILD_CACHE:
        return _BUILD_CACHE["nc"]

    import concourse.bacc as bacc
    import concourse.mybir as mybir
    import concourse.tile as tile
    from concourse.tile_rust import add_dep_helper

    f32 = mybir.dt.float32
    f32r = mybir.dt.float32r
    AF = mybir.ActivationFunctionType
    ADD = mybir.AluOpType.add
    MULT = mybir.AluOpType.mult

    nc = bacc.Bacc("TRN2", target_bir_lowering=False)
    xT_d = nc.dram_tensor("xT", [D, S], f32, kind="ExternalInput")
    wq_d = nc.dram_tensor("wqT", [D, DG], f32, kind="ExternalInput")
    wk_d = nc.dram_tensor("wkT", [D, DG], f32, kind="ExternalInput")
    wv_d = nc.dram_tensor("wvT", [D, DG], f32, kind="ExternalInput")
    wo_d = nc.dram_tensor("woT", [DG, D], f32, kind="ExternalInput")
    mask_d = nc.dram_tensor("mask", [128, 128], f32, kind="ExternalInput")
    ones_d = nc.dram_tensor("onesb", [128, 128], f32, kind="ExternalInput")
    out_d = nc.dram_tensor("out", [S, D], f32, kind="ExternalOutput")

    with tile.TileContext(nc) as tc:
        with tc.tile_pool(name="persist", bufs=1) as pp:
            QT = pp.tile([128, 4, S], f32r, tag="QT")
            KT = pp.tile([128, 4, S], f32r, tag="KT")
            V = pp.tile([128, 16, HG, DK + 1], f32r, tag="V")
            maskb = pp.tile([128, 128], f32, tag="maskb")
            onesb = pp.tile([128, 128], f32, tag="onesb")
            ones_r = pp.tile([128, 64], f32r, tag="ones_r")
            nc.sync.dma_start(maskb[:], mask_d[:, :])
            nc.sync.dma_start(onesb[:], ones_d[:, :])
            nc.sync.dma_start(ones_r[:], ones_d[:, 0:64].bitcast(f32r))
            # ones column of V (f32 -> f32r rounding copy)
            nc.vector.tensor_copy(
                V[:, :, :, DK : DK + 1],
                onesb[:, 0:128].rearrange("p (so h) -> p so h", so=16)[:, :, :, None],
            )

            # ---------------- stage 1: projections ----------------
            with (
                tc.tile_pool(name="stage1", bufs=1) as s1p,
                tc.tile_pool(name="ps1", bufs=1, space="PSUM") as ps1,
            ):
                wq = s1p.tile([128, CO, DG], f32r, tag="wq")
                wk = s1p.tile([128, CO, DG], f32r, tag="wk")
                wv = s1p.tile([128, CO, DG], f32r, tag="wv")
                nc.sync.dma_start(
                    wq, wq_d[:, :].rearrange("(co ci) d -> ci co d", ci=128).bitcast(f32r)
                )
                nc.sync.dma_start(
                    wk, wk_d[:, :].rearrange("(co ci) d -> ci co d", ci=128).bitcast(f32r)
                )
                nc.sync.dma_start(
                    wv, wv_d[:, :].rearrange("(co ci) d -> ci co d", ci=128).bitcast(f32r)
                )
                for sb in range(SBLK):
                    xt = s1p.tile([128, CO, 512], f32r, tag="xt", bufs=2)
                    nc.sync.dma_start(
                        xt,
                        xT_d[:, sb * 512 : (sb + 1) * 512]
                        .rearrange("(co ci) s -> ci co s", ci=128)
                        .bitcast(f32r),
                    )
                    ssl = slice(sb * 512, (sb + 1) * 512)
                    for do in range(4):
                        dsl = slice(do * 128, (do + 1) * 128)
                        pq = ps1.tile([128, 512], f32, tag="proj", bufs=4, name=f"pq{sb}{do}")
                        for co in range(CO):
                            nc.tensor.matmul(
                                pq, wq[:, co, dsl], xt[:, co, :],
                                start=(co == 0), stop=(co == CO - 1),
                            )
                        nc.any.tensor_copy(QT[:, do, ssl], pq[:])
                        pk = ps1.tile([128, 512], f32, tag="proj", bufs=4, name=f"pk{sb}{do}")
                        for co in range(CO):
                            nc.tensor.matmul(
                                pk, wk[:, co, dsl], xt[:, co, :],
                                start=(co == 0), stop=(co == CO - 1),
                            )
                        nc.any.tensor_copy(KT[:, do, ssl], pk[:])
                    for so in range(4):
                        pv = ps1.tile([128, 512], f32, tag="proj", bufs=4, name=f"pv{sb}{so}")
                        for co in range(CO):
                            nc.tensor.matmul(
                                pv, xt[:, co, so * 128 : (so + 1) * 128], wv[:, co, :],
                                start=(co == 0), stop=(co == CO - 1),
                            )
                        nc.any.tensor_copy(
                            V[:, sb * 4 + so, :, 0:DK],
                            pv[:].rearrange("p (h d) -> p h d", h=HG),
                        )

            # ---------------- stages 2+3: attention + output ----------------
            with (
                tc.tile_pool(name="atp", bufs=1) as atp,
                tc.tile_pool(name="stage2", bufs=1) as s2p,
                tc.tile_pool(name="ps2", bufs=1, space="PSUM") as ps2,
            ):
                AT = atp.tile([128, 4, S], f32r, tag="AT")
                wo = s2p.tile([128, 4, D], f32r, tag="wo")
                nc.sync.dma_start(
                    wo,
                    wo_d[:, :].rearrange("(io ip) j -> ip io j", ip=128).bitcast(f32r),
                )
                pe_prev = [None]  # last instr of the previous PE group

                def pe_group(insts):
                    # force PE issue order: first of this group after last of
                    # the previous group; chain within the group
                    if not insts:
                        return
                    if pe_prev[0] is not None:
                        add_dep_helper(
                            insts[0].ins, pe_prev[0].ins, sync=False,
                            reason="pe group order",
                        )
                    for a, b in zip(insts[1:], insts):
                        add_dep_helper(a.ins, b.ins, sync=False, reason="pe chain")
                    pe_prev[0] = insts[-1]

                for qb in range(QB):
                    qsl = slice(qb * 512, (qb + 1) * 512)
                    nkb = 4 * qb + 4
                    # denominator gather tiles; rows at pair*32 (SBUF
                    # partition offsets must be 32-aligned).  memset to 1.0
                    # so the batched reciprocal sees no garbage lanes.
                    sums_a = s2p.tile([128, 512], f32, tag="sums_a", bufs=2, name=f"sums_a{qb}")
                    sums_b = s2p.tile([128, 512], f32, tag="sums_b", bufs=2, name=f"sums_b{qb}")
                    nc.gpsimd.memset(sums_a[:], 1.0)
                    nc.gpsimd.memset(sums_b[:], 1.0)
                    for pair in range(4):
                        heads = (2 * pair, 2 * pair + 1)
                        sums_of = {heads[0]: sums_a, heads[1]: sums_b}
                        ovs = {}
                        for h in heads:
                            ov = ps2.tile(
                                [DK + 1, 512], f32, tag="ovpo", bufs=2, name=f"ov{h}q{qb}"
                            )
                            ovs[h] = ov
                        # chunked kb loop with one-chunk lag: emit a
                        # chunk of scores+exps, then the PREVIOUS chunk's
                        # attnV matmuls grouped per head (consecutive
                        # same-bank accumulation -- interleaving score and
                        # attnV matmuls forces a PE pipeline drain at every
                        # lhsT row-group conflict, measured 733 vs 237 ns/mm)
                        def emit_avs(items):
                            grp = []
                            for h in heads:
                                for (pkb, pcs, pets) in items:
                                    grp.append(nc.tensor.matmul(
                                        ovs[h][:, pcs:], V[:, pkb, h, :],
                                        pets[h][:, pcs:],
                                        start=(pkb == 0), stop=(pkb == nkb - 1),
                                    ))
                            pe_group(grp)

                        CH = 3
                        kbs = list(range(nkb))
                        chunks = [kbs[i : i + CH] for i in range(0, nkb, CH)]
                        pend = None
                        for chunk in chunks:
                            items = []
                            sc_grp = []
                            for kb in chunk:
                                ksl = slice(kb * 128, (kb + 1) * 128)
                                d = kb - 4 * qb
                                cs = 128 * d if d > 0 else 0
                                ets = {}
                                for h in heads:
                                    base = 64 * (h % 2)
                                    psl = slice(base, base + 64)
                                    sp = ps2.tile(
                                        [128, 512], f32, tag="score", bufs=6,
                                        name=f"sp{h}q{qb}k{kb}",
                                    )
                                    sc_grp.append(nc.tensor.matmul(
                                        sp, KT[psl, pair, ksl], QT[psl, pair, qsl],
                                        start=True, stop=True,
                                    ))
                                    if d >= 0:
                                        nc.vector.tensor_tensor(
                                            sp[:, cs : cs + 128],
                                            sp[:, cs : cs + 128],
                                            maskb[:, :],
                                            ADD,
                                        )
                                    et = s2p.tile(
                                        [128, 512], f32r, tag="et", bufs=12,
                                        name=f"et{h}q{qb}k{kb}",
                                    )
                                    nc.scalar.activation(
                                        et[:, cs:], sp[:, cs:], AF.Exp, scale=0.125
                                    )
                                    ets[h] = et
                                items.append((kb, cs, ets))
                            pe_group(sc_grp)
                            if pend is not None:
                                emit_avs(pend)
                            pend = items
                        emit_avs(pend)
                        for h in heads:
                            base = 64 * (h % 2)
                            ov = ovs[h]
                            # gather denominator row; write unnormalized AT
                            nc.vector.tensor_copy(
                                sums_of[h][pair * 32 : pair * 32 + 1, :],
                                ov[DK : DK + 1, :],
                            )
                            nc.any.tensor_copy(
                                AT[base : base + 64, pair, qsl], ov[0:DK, :]
                            )
                    # normalize all 8 heads of this q-block: batched
                    # reciprocal, then per head a K=1 broadcast matmul and an
                    # in-place multiply on AT.
                    srec_a = s2p.tile([128, 512], f32r, tag="srec_a", bufs=2, name=f"srec_a{qb}")
                    srec_b = s2p.tile([128, 512], f32r, tag="srec_b", bufs=2, name=f"srec_b{qb}")
                    with nc.allow_low_precision(reason="f32r is fp32-width"):
                        nc.vector.reciprocal(srec_a[:], sums_a[:])
                        nc.vector.reciprocal(srec_b[:], sums_b[:])
                    rbs_ = {}
                    rb_grp = []
                    for pair in range(4):
                        heads = (2 * pair, 2 * pair + 1)
                        srec_of = {heads[0]: srec_a, heads[1]: srec_b}
                        for h in heads:
                            p32 = pair * 32
                            rb = ps2.tile([128, 512], f32, tag="score", bufs=6, name=f"rb{h}{qb}")
                            rb_grp.append(nc.tensor.matmul(
                                rb[0:64, :],
                                ones_r[p32 : p32 + 1, :],
                                srec_of[h][p32 : p32 + 1, :],
                                start=True, stop=True,
                                tile_position=(p32, 0) if p32 == 96 else None,
                            ))
                            rbs_[h] = rb
                    pe_group(rb_grp)
                    for pair in range(4):
                        for h in (2 * pair, 2 * pair + 1):
                            base = 64 * (h % 2)
                            nc.vector.tensor_tensor(
                                AT[base : base + 64, pair, qsl],
                                AT[base : base + 64, pair, qsl].bitcast(f32),
                                rbs_[h][0:64, :],
                                MULT,
                            )
                    # output projection for the 4 finished 128-row s-chunks
                    for sc in range(4 * qb, 4 * qb + 4):
                        og = s2p.tile([128, D], f32, tag="og", bufs=2, name=f"og{sc}")
                        po_grp = []
                        for jh in range(2):
                            po = ps2.tile(
                                [128, 512], f32, tag="ovpo", bufs=2, name=f"po{sc}{jh}"
                            )
                            for io in range(4):
                                po_grp.append(nc.tensor.matmul(
                                    po,
                                    AT[:, io, sc * 128 : (sc + 1) * 128],
                                    wo[:, io, jh * 512 : (jh + 1) * 512],
                                    start=(io == 0), stop=(io == 3),
                                ))
                            nc.any.tensor_copy(og[:, jh * 512 : (jh + 1) * 512], po[:])
                        pe_group(po_grp)
                        nc.sync.dma_start(out_d[sc * 128 : (sc + 1) * 128, :], og[:])

    nc.compile()
    _BUILD_CACHE["nc"] = nc
    return nc


def _host_inputs(x, WQ, WK, WV, WO):
    ki = np.arange(128, dtype=np.float32)[:, None]
    qj = np.arange(128, dtype=np.float32)[None, :]
    # stripe mask: within the partial 128-col stripe of diagonal chunk d,
    # allowed iff (qj - 128d) >= ki, i.e. local column >= ki.
    mask = np.where(qj >= ki, 0.0, NEG).astype(np.float32)
    onesb = np.ones((128, 128), dtype=np.float32)

    in_maps = []
    for b in range(B):
        xT = np.ascontiguousarray(x[b].T)
        for g in range(G):
            sl = slice(g * DG, (g + 1) * DG)
            in_maps.append(
                {
                    "xT": xT,
                    "wqT": np.ascontiguousarray(WQ[sl, :].T),
                    "wkT": np.ascontiguousarray(WK[sl, :].T),
                    "wvT": np.ascontiguousarray(WV[sl, :].T),
                    "woT": np.ascontiguousarray(WO[:, sl].T),
                    "mask": mask,
                    "onesb": onesb,
                }
            )
    return in_maps


def kernel(x, WQ, WK, WV, WO):
    from concourse.bass_utils import run_bass_kernel_spmd

    x = np.asarray(x, dtype=np.float32)
    WQ = np.asarray(WQ, dtype=np.float32)
    WK = np.asarray(WK, dtype=np.float32)
    WV = np.asarray(WV, dtype=np.float32)
    WO = np.asarray(WO, dtype=np.float32)

    nc = _build()
    in_maps = _host_inputs(x, WQ, WK, WV, WO)
    res = run_bass_kernel_spmd(
        nc,
        in_maps,
        core_ids=list(range(8)),
        trace=bool(os.environ.get("KERNEL_TRACE")),
    )
    kernel.last_results = res
    parts = [r["out"] for r in res.results]
    out = np.stack([parts[2 * b] + parts[2 * b + 1] for b in range(B)], axis=0)
    return out.astype(np.float32)


# revision 11
# speedup vs baseline: 1.4637x; 1.0104x over previous
"""Causal multi-head self-attention on 8 Trainium2 NeuronCores.

Sharding: core = (batch b, head-group g).  B=4 batches x 2 groups of 8 heads
= 8 cores.  Each core computes Q/K/V projections for its 8 heads, causal
attention, and a partial output projection (row-shard of WO); the host sums
the two partials per batch (the tensor-parallel all-reduce, done at gather).

Per-core device pipeline (all matmuls in float32r = full fp32 precision via
the PE's 2-pass mode, 1 cycle/row at N>=256):
  stage 1: QT[d',s], KT[d',s] (transposed) and V[s,d'] (natural) projections
           from host-pre-transposed xT and weight shards.
  stage 2: q-block outer loop; per (q-block, head-pair): scoresT[k,q] per
           128-wide k-chunk, causal mask add on the single partial 128-col
           stripe of diagonal chunks, exp over the allowed column range only
           (no max subtraction -- scores are O(5) so exp is safe in fp32),
           attn@V with a ones column appended to V so PSUM row 64
           accumulates the softmax denominator.  The kb loop is software
           pipelined (attnV lags scores by one chunk) so the in-order PE
           stream never stalls on the ACT exp.  Denominators gather into
           [128, 512] tiles (rows at pair*32) so reciprocals run 8 rows at
           a time; normalization is an in-place multiply on AT.
  stage 3: output projection for the finished q rows, interleaved with the
           next q-block's attention.
"""

import os
import numpy as np

B, S, D = 4, 2048, 1024
H_TOTAL, DK = 16, 64
G = 2          # head groups (cores per batch)
HG = 8         # heads per core
DG = 512       # head dims per core
CO = 8         # contraction chunks of 128 over D
SBLK = 4       # 512-wide s blocks
QB = 4         # 512-wide q blocks
NEG = -1e9

_BUILD_CACHE = {}


def _build():
    if "nc" in _BUILD_CACHE:
        return _BUILD_CACHE["nc"]

    import concourse.bacc as bacc
    import concourse.mybir as mybir
    import concourse.tile as tile
    from concourse.tile_rust import add_dep_helper

    f32 = mybir.dt.float32
    f32r = mybir.dt.float32r
    AF = mybir.ActivationFunctionType
    ADD = mybir.AluOpType.add
    MULT = mybir.AluOpType.mult

    nc = bacc.Bacc("TRN2", target_bir_lowering=False)
    xT_d = nc.dram_tensor("xT", [D, S], f32, kind="ExternalInput")
    wq_d = nc.dram_tensor("wqT", [D, DG], f32, kind="ExternalInput")
    wk_d = nc.dram_tensor("wkT", [D, DG], f32, kind="ExternalInput")
    wv_d = nc.dram_tensor("wvT", [D, DG], f32, kind="ExternalInput")
    wo_d = nc.dram_tensor("woT", [DG, D], f32, kind="ExternalInput")
    mask_d = nc.dram_tensor("mask", [128, 128], f32, kind="ExternalInput")
    ones_d = nc.dram_tensor("onesb", [128, 128], f32, kind="ExternalInput")
    out_d = nc.dram_tensor("out", [S, D], f32, kind="ExternalOutput")

    with tile.TileContext(nc) as tc:
        with tc.tile_pool(name="persist", bufs=1) as pp:
            QT = pp.tile([128, 4, S], f32r, tag="QT")
            KT = pp.tile([128, 4, S], f32r, tag="KT")
            V = pp.tile([128, 16, HG, DK + 1], f32r, tag="V")
            maskb = pp.tile([128, 128], f32, tag="maskb")
            onesb = pp.tile([128, 128], f32, tag="onesb")
            ones_r = pp.tile([128, 64], f32r, tag="ones_r")
            nc.sync.dma_start(maskb[:], mask_d[:, :])
            nc.sync.dma_start(onesb[:], ones_d[:, :])
            nc.sync.dma_start(ones_r[:], ones_d[:, 0:64].bitcast(f32r))
            # ones column of V (f32 -> f32r rounding copy)
            nc.vector.tensor_copy(
                V[:, :, :, DK : DK + 1],
                onesb[:, 0:128].rearrange("p (so h) -> p so h", so=16)[:, :, :, None],
            )

            # ---------------- stage 1: projections ----------------
            with (
                tc.tile_pool(name="stage1", bufs=1) as s1p,
                tc.tile_pool(name="ps1", bufs=1, space="PSUM") as ps1,
            ):
                wq = s1p.tile([128, CO, DG], f32r, tag="wq")
                wk = s1p.tile([128, CO, DG], f32r, tag="wk")
                wv = s1p.tile([128, CO, DG], f32r, tag="wv")
                nc.sync.dma_start(
                    wq, wq_d[:, :].rearrange("(co ci) d -> ci co d", ci=128).bitcast(f32r)
                )
                nc.sync.dma_start(
                    wk, wk_d[:, :].rearrange("(co ci) d -> ci co d", ci=128).bitcast(f32r)
                )
                nc.sync.dma_start(
                    wv, wv_d[:, :].rearrange("(co ci) d -> ci co d", ci=128).bitcast(f32r)
                )
                for sb in range(SBLK):
                    xt = s1p.tile([128, CO, 512], f32r, tag="xt", bufs=2)
                    nc.sync.dma_start(
                        xt,
                        xT_d[:, sb * 512 : (sb + 1) * 512]
                        .rearrange("(co ci) s -> ci co s", ci=128)
                        .bitcast(f32r),
                    )
                    ssl = slice(sb * 512, (sb + 1) * 512)
                    for do in range(4):
                        dsl = slice(do * 128, (do + 1) * 128)
                        pq = ps1.tile([128, 512], f32, tag="proj", bufs=4, name=f"pq{sb}{do}")
                        for co in range(CO):
                            nc.tensor.matmul(
                                pq, wq[:, co, dsl], xt[:, co, :],
                                start=(co == 0), stop=(co == CO - 1),
                            )
                        nc.any.tensor_copy(QT[:, do, ssl], pq[:])
                        pk = ps1.tile([128, 512], f32, tag="proj", bufs=4, name=f"pk{sb}{do}")
                        for co in range(CO):
                            nc.tensor.matmul(
                                pk, wk[:, co, dsl], xt[:, co, :],
                                start=(co == 0), stop=(co == CO - 1),
                            )
                        nc.any.tensor_copy(KT[:, do, ssl], pk[:])
                    for so in range(4):
                        pv = ps1.tile([128, 512], f32, tag="proj", bufs=4, name=f"pv{sb}{so}")
                        for co in range(CO):
                            nc.tensor.matmul(
                                pv, xt[:, co, so * 128 : (so + 1) * 128], wv[:, co, :],
                                start=(co == 0), stop=(co == CO - 1),
                            )
                        nc.any.tensor_copy(
                            V[:, sb * 4 + so, :, 0:DK],
                            pv[:].rearrange("p (h d) -> p h d", h=HG),
                        )

            # ---------------- stages 2+3: attention + output ----------------
            with (
                tc.tile_pool(name="atp", bufs=1) as atp,
                tc.tile_pool(name="stage2", bufs=1) as s2p,
                tc.tile_pool(name="ps2", bufs=1, space="PSUM") as ps2,
            ):
                AT = atp.tile([128, 4, S], f32r, tag="AT")
                wo = s2p.tile([128, 4, D], f32r, tag="wo")
                nc.sync.dma_start(
                    wo,
                    wo_d[:, :].rearrange("(io ip) j -> ip io j", ip=128).bitcast(f32r),
                )
                pe_prev = [None]  # last instr of the previous PE group

                def pe_group(insts):
                    # force PE issue order: first of this group after last of
                    # the previous group; chain within the group
                    if not insts:
                        return
                    if pe_prev[0] is not None:
                        add_dep_helper(
                            insts[0].ins, pe_prev[0].ins, sync=False,
                            reason="pe group order",
                        )
                    for a, b in zip(insts[1:], insts):
                        add_dep_helper(a.ins, b.ins, sync=False, reason="pe chain")
                    pe_prev[0] = insts[-1]

                for qb in range(QB):
                    qsl = slice(qb * 512, (qb + 1) * 512)
                    nkb = 4 * qb + 4
                    # denominator gather tiles; rows at pair*32 (SBUF
                    # partition offsets must be 32-aligned).  memset to 1.0
                    # so the batched reciprocal sees no garbage lanes.
                    sums_a = s2p.tile([128, 512], f32, tag="sums_a", bufs=2, name=f"sums_a{qb}")
                    sums_b = s2p.tile([128, 512], f32, tag="sums_b", bufs=2, name=f"sums_b{qb}")
                    nc.gpsimd.memset(sums_a[:], 1.0)
                    nc.gpsimd.memset(sums_b[:], 1.0)
                    for pair in range(4):
                        heads = (2 * pair, 2 * pair + 1)
                        sums_of = {heads[0]: sums_a, heads[1]: sums_b}
                        ovs = {}
                        for h in heads:
                            ov = ps2.tile(
                                [DK + 1, 512], f32, tag="ovpo", bufs=2, name=f"ov{h}q{qb}"
                            )
                            ovs[h] = ov
                        # chunked kb loop with one-chunk lag: emit a
                        # chunk of scores+exps, then the PREVIOUS chunk's
                        # attnV matmuls grouped per head (consecutive
                        # same-bank accumulation -- interleaving score and
                        # attnV matmuls forces a PE pipeline drain at every
                        # lhsT row-group conflict, measured 733 vs 237 ns/mm)
                        def emit_avs(items):
                            grp = []
                            for h in heads:
                                for (pkb, pcs, pets) in items:
                                    grp.append(nc.tensor.matmul(
                                        ovs[h][:, pcs:], V[:, pkb, h, :],
                                        pets[h][:, pcs:],
                                        start=(pkb == 0), stop=(pkb == nkb - 1),
                                    ))
                            pe_group(grp)

                        CH = 3
                        kbs = list(range(nkb))
                        chunks = [kbs[i : i + CH] for i in range(0, nkb, CH)]
                        pend = None
                        for chunk in chunks:
                            items = []
                            sc_grp = []
                            for kb in chunk:
                                ksl = slice(kb * 128, (kb + 1) * 128)
                                d = kb - 4 * qb
                                cs = 128 * d if d > 0 else 0
                                ets = {}
                                for h in heads:
                                    base = 64 * (h % 2)
                                    psl = slice(base, base + 64)
                                    sp = ps2.tile(
                                        [128, 512], f32, tag="score", bufs=6,
                                        name=f"sp{h}q{qb}k{kb}",
                                    )
                                    sc_grp.append(nc.tensor.matmul(
                                        sp, KT[psl, pair, ksl], QT[psl, pair, qsl],
                                        start=True, stop=True,
                                    ))
                                    if d >= 0:
                                        nc.vector.tensor_tensor(
                                            sp[:, cs : cs + 128],
                                            sp[:, cs : cs + 128],
                                            maskb[:, :],
                                            ADD,
                                        )
                                    et = s2p.tile(
                                        [128, 512], f32r, tag="et", bufs=12,
                                        name=f"et{h}q{qb}k{kb}",
                                    )
                                    nc.scalar.activation(
                                        et[:, cs:], sp[:, cs:], AF.Exp, scale=0.125
                                    )
                                    ets[h] = et
                                items.append((kb, cs, ets))
                            pe_group(sc_grp)
                            if pend is not None:
                                emit_avs(pend)
                            pend = items
                        emit_avs(pend)
                        for h in heads:
                            base = 64 * (h % 2)
                            ov = ovs[h]
                            # gather denominator row; write unnormalized AT
                            nc.vector.tensor_copy(
                                sums_of[h][pair * 32 : pair * 32 + 1, :],
                                ov[DK : DK + 1, :],
                            )
                            nc.any.tensor_copy(
                                AT[base : base + 64, pair, qsl], ov[0:DK, :]
                            )
                    # normalize all 8 heads of this q-block: batched
                    # reciprocal, then per head a K=1 broadcast matmul and an
                    # in-place multiply on AT.
                    srec_a = s2p.tile([128, 512], f32r, tag="srec_a", bufs=2, name=f"srec_a{qb}")
                    srec_b = s2p.tile([128, 512], f32r, tag="srec_b", bufs=2, name=f"srec_b{qb}")
                    with nc.allow_low_precision(reason="f32r is fp32-width"):
                        nc.vector.reciprocal(srec_a[:], sums_a[:])
                        nc.vector.reciprocal(srec_b[:], sums_b[:])
                    rbs_ = {}
                    rb_grp = []
                    for pair in range(4):
                        heads = (2 * pair, 2 * pair + 1)
                        srec_of = {heads[0]: srec_a, heads[1]: srec_b}
                        for h in heads:
                            p32 = pair * 32
                            rb = ps2.tile([128, 512], f32, tag="score", bufs=6, name=f"rb{h}{qb}")
                            rb_grp.append(nc.tensor.matmul(
                                rb[0:64, :],
                                ones_r[p32 : p32 + 1, :],
                                srec_of[h][p32 : p32 + 1, :],
                                start=True, stop=True,
                                tile_position=(p32, 0) if p32 == 96 else None,
                            ))
                            rbs_[h] = rb
                    pe_group(rb_grp)
                    for pair in range(4):
                        for h in (2 * pair, 2 * pair + 1):
                            base = 64 * (h % 2)
                            nc.vector.tensor_tensor(
                                AT[base : base + 64, pair, qsl],
                                AT[base : base + 64, pair, qsl].bitcast(f32),
                                rbs_[h][0:64, :],
                                MULT,
                            )
                    # output projection for the 4 finished 128-row s-chunks
                    for sc in range(4 * qb, 4 * qb + 4):
                        og = s2p.tile([128, D], f32, tag="og", bufs=2, name=f"og{sc}")
                        po_grp = []
                        for jh in range(2):
                            po = ps2.tile(
                                [128, 512], f32, tag="ovpo", bufs=2, name=f"po{sc}{jh}"
                            )
                            for io in range(4):
                                po_grp.append(nc.tensor.matmul(
                                    po,
                                    AT[:, io, sc * 128 : (sc + 1) * 128],
                                    wo[:, io, jh * 512 : (jh + 1) * 512],
                                    start=(io == 0), stop=(io == 3),
                                ))
                            nc.any.tensor_copy(og[:, jh * 512 : (jh + 1) * 512], po[:])
                        pe_group(po_grp)
                        nc.sync.dma_start(out_d[sc * 128 : (sc + 1) * 128, :], og[:])

    nc.compile()
    _BUILD_CACHE["nc"] = nc
    return nc


def _host_inputs(x, WQ, WK, WV, WO):
    ki = np.arange(128, dtype=np.float32)[:, None]
    qj = np.arange(128, dtype=np.float32)[None, :]
    # stripe mask: within the partial 128-col stripe of diagonal chunk d,
    # allowed iff (qj - 128d) >= ki, i.e. local column >= ki.
    mask = np.where(qj >= ki, 0.0, NEG).astype(np.float32)
    onesb = np.ones((128, 128), dtype=np.float32)

    in_maps = []
    for b in range(B):
        xT = np.ascontiguousarray(x[b].T)
        for g in range(G):
            sl = slice(g * DG, (g + 1) * DG)
            in_maps.append(
                {
                    "xT": xT,
                    "wqT": np.ascontiguousarray(WQ[sl, :].T),
                    "wkT": np.ascontiguousarray(WK[sl, :].T),
                    "wvT": np.ascontiguousarray(WV[sl, :].T),
                    "woT": np.ascontiguousarray(WO[:, sl].T),
                    "mask": mask,
                    "onesb": onesb,
                }
            )
    return in_maps


def kernel(x, WQ, WK, WV, WO):
    from concourse.bass_utils import run_bass_kernel_spmd

    x = np.asarray(x, dtype=np.float32)
    WQ = np.asarray(WQ, dtype=np.float32)
    WK = np.asarray(WK, dtype=np.float32)
    WV = np.asarray(WV, dtype=np.float32)
    WO = np.asarray(WO, dtype=np.float32)

    nc = _build()
    in_maps = _host_inputs(x, WQ, WK, WV, WO)
    res = run_bass_kernel_spmd(
        nc,
        in_maps,
        core_ids=list(range(8)),
        trace=bool(os.environ.get("KERNEL_TRACE")),
    )
    kernel.last_results = res
    parts = [r["out"] for r in res.results]
    out = np.stack([parts[2 * b] + parts[2 * b + 1] for b in range(B)], axis=0)
    return out.astype(np.float32)


# revision 12
# speedup vs baseline: 1.4866x; 1.0157x over previous
"""Causal multi-head self-attention on 8 Trainium2 NeuronCores.

Sharding: core = (batch b, head-group g).  B=4 batches x 2 groups of 8 heads
= 8 cores.  Each core computes Q/K/V projections for its 8 heads, causal
attention, and a partial output projection (row-shard of WO); the host sums
the two partials per batch (the tensor-parallel all-reduce, done at gather).

Per-core device pipeline (all matmuls in float32r = full fp32 precision via
the PE's 2-pass mode, 1 cycle/row at N>=256):
  stage 1: QT[d',s], KT[d',s] (transposed) and V[s,d'] (natural) projections
           from host-pre-transposed xT and weight shards.
  stage 2: q-block outer loop; per (q-block, head-pair): scoresT[k,q] per
           128-wide k-chunk, causal mask add on the single partial 128-col
           stripe of diagonal chunks, exp over the allowed column range only
           (no max subtraction -- scores are O(5) so exp is safe in fp32),
           attn@V with a ones column appended to V so PSUM row 64
           accumulates the softmax denominator.  The kb loop is software
           pipelined (attnV lags scores by one chunk) so the in-order PE
           stream never stalls on the ACT exp.  Denominators gather into
           [128, 512] tiles (rows at pair*32) so reciprocals run 8 rows at
           a time; normalization is an in-place multiply on AT.
  stage 3: output projection for the finished q rows, interleaved with the
           next q-block's attention.
"""

import os
import numpy as np

B, S, D = 4, 2048, 1024
H_TOTAL, DK = 16, 64
G = 2          # head groups (cores per batch)
HG = 8         # heads per core
DG = 512       # head dims per core
CO = 8         # contraction chunks of 128 over D
SBLK = 4       # 512-wide s blocks
QB = 4         # 512-wide q blocks
NEG = -1e9

_BUILD_CACHE = {}


def _build():
    if "nc" in _BUILD_CACHE:
        return _BUILD_CACHE["nc"]

    import concourse.bacc as bacc
    import concourse.mybir as mybir
    import concourse.tile as tile
    from concourse.tile_rust import add_dep_helper

    f32 = mybir.dt.float32
    f32r = mybir.dt.float32r
    AF = mybir.ActivationFunctionType
    ADD = mybir.AluOpType.add
    MULT = mybir.AluOpType.mult

    nc = bacc.Bacc("TRN2", target_bir_lowering=False)
    xT_d = nc.dram_tensor("xT", [D, S], f32, kind="ExternalInput")
    wq_d = nc.dram_tensor("wqT", [D, DG], f32, kind="ExternalInput")
    wk_d = nc.dram_tensor("wkT", [D, DG], f32, kind="ExternalInput")
    wv_d = nc.dram_tensor("wvT", [D, DG], f32, kind="ExternalInput")
    wo_d = nc.dram_tensor("woT", [DG, D], f32, kind="ExternalInput")
    mask_d = nc.dram_tensor("mask", [128, 128], f32, kind="ExternalInput")
    ones_d = nc.dram_tensor("onesb", [128, 128], f32, kind="ExternalInput")
    out_d = nc.dram_tensor("out", [S, D], f32, kind="ExternalOutput")

    with tile.TileContext(nc) as tc:
        with tc.tile_pool(name="persist", bufs=1) as pp:
            QT = pp.tile([128, 4, S], f32r, tag="QT")
            KT = pp.tile([128, 4, S], f32r, tag="KT")
            V = pp.tile([128, 16, HG, DK + 1], f32r, tag="V")
            maskb = pp.tile([128, 128], f32, tag="maskb")
            onesb = pp.tile([128, 128], f32, tag="onesb")
            ones_r = pp.tile([128, 64], f32r, tag="ones_r")
            nc.sync.dma_start(maskb[:], mask_d[:, :])
            nc.sync.dma_start(onesb[:], ones_d[:, :])
            nc.sync.dma_start(ones_r[:], ones_d[:, 0:64].bitcast(f32r))
            # ones column of V (f32 -> f32r rounding copy)
            nc.vector.tensor_copy(
                V[:, :, :, DK : DK + 1],
                onesb[:, 0:128].rearrange("p (so h) -> p so h", so=16)[:, :, :, None],
            )

            # ---------------- stage 1: projections ----------------
            with (
                tc.tile_pool(name="stage1", bufs=1) as s1p,
                tc.tile_pool(name="ps1", bufs=1, space="PSUM") as ps1,
            ):
                wq = s1p.tile([128, CO, DG], f32r, tag="wq")
                wk = s1p.tile([128, CO, DG], f32r, tag="wk")
                wv = s1p.tile([128, CO, DG], f32r, tag="wv")
                nc.sync.dma_start(
                    wq, wq_d[:, :].rearrange("(co ci) d -> ci co d", ci=128).bitcast(f32r)
                )
                nc.sync.dma_start(
                    wk, wk_d[:, :].rearrange("(co ci) d -> ci co d", ci=128).bitcast(f32r)
                )
                nc.sync.dma_start(
                    wv, wv_d[:, :].rearrange("(co ci) d -> ci co d", ci=128).bitcast(f32r)
                )
                for sb in range(SBLK):
                    xt = s1p.tile([128, CO, 512], f32r, tag="xt", bufs=2)
                    nc.sync.dma_start(
                        xt,
                        xT_d[:, sb * 512 : (sb + 1) * 512]
                        .rearrange("(co ci) s -> ci co s", ci=128)
                        .bitcast(f32r),
                    )
                    ssl = slice(sb * 512, (sb + 1) * 512)
                    for do in range(4):
                        dsl = slice(do * 128, (do + 1) * 128)
                        pq = ps1.tile([128, 512], f32, tag="proj", bufs=4, name=f"pq{sb}{do}")
                        for co in range(CO):
                            nc.tensor.matmul(
                                pq, wq[:, co, dsl], xt[:, co, :],
                                start=(co == 0), stop=(co == CO - 1),
                            )
                        nc.any.tensor_copy(QT[:, do, ssl], pq[:])
                        pk = ps1.tile([128, 512], f32, tag="proj", bufs=4, name=f"pk{sb}{do}")
                        for co in range(CO):
                            nc.tensor.matmul(
                                pk, wk[:, co, dsl], xt[:, co, :],
                                start=(co == 0), stop=(co == CO - 1),
                            )
                        nc.any.tensor_copy(KT[:, do, ssl], pk[:])
                    for so in range(4):
                        pv = ps1.tile([128, 512], f32, tag="proj", bufs=4, name=f"pv{sb}{so}")
                        for co in range(CO):
                            nc.tensor.matmul(
                                pv, xt[:, co, so * 128 : (so + 1) * 128], wv[:, co, :],
                                start=(co == 0), stop=(co == CO - 1),
                            )
                        nc.any.tensor_copy(
                            V[:, sb * 4 + so, :, 0:DK],
                            pv[:].rearrange("p (h d) -> p h d", h=HG),
                        )

            # ---------------- stages 2+3: attention + output ----------------
            with (
                tc.tile_pool(name="atp", bufs=1) as atp,
                tc.tile_pool(name="stage2", bufs=1) as s2p,
                tc.tile_pool(name="ps2", bufs=1, space="PSUM") as ps2,
            ):
                AT = atp.tile([128, 4, S], f32r, tag="AT")
                wo = s2p.tile([128, 4, D], f32r, tag="wo")
                nc.sync.dma_start(
                    wo,
                    wo_d[:, :].rearrange("(io ip) j -> ip io j", ip=128).bitcast(f32r),
                )
                pe_prev = [None]  # last instr of the previous PE group

                def pe_group(insts):
                    # force PE issue order: first of this group after last of
                    # the previous group; chain within the group
                    if not insts:
                        return
                    if pe_prev[0] is not None:
                        add_dep_helper(
                            insts[0].ins, pe_prev[0].ins, sync=False,
                            reason="pe group order",
                        )
                    for a, b in zip(insts[1:], insts):
                        add_dep_helper(a.ins, b.ins, sync=False, reason="pe chain")
                    pe_prev[0] = insts[-1]

                deferred_fin = [None]

                for qb in range(QB):
                    qsl = slice(qb * 512, (qb + 1) * 512)
                    nkb = 4 * qb + 4
                    # denominator gather tiles; rows at pair*32 (SBUF
                    # partition offsets must be 32-aligned).  memset to 1.0
                    # so the batched reciprocal sees no garbage lanes.
                    sums_a = s2p.tile([128, 512], f32, tag="sums_a", bufs=2, name=f"sums_a{qb}")
                    sums_b = s2p.tile([128, 512], f32, tag="sums_b", bufs=2, name=f"sums_b{qb}")
                    nc.gpsimd.memset(sums_a[:], 1.0)
                    nc.gpsimd.memset(sums_b[:], 1.0)
                    for pair in range(4):
                        if pair == 1 and deferred_fin[0] is not None:
                            # fire the previous q-block's normalization +
                            # output projection here: pair 0's ov slots are
                            # released and pair 1's not yet allocated, so the
                            # deferred po tiles take clean ovpo ring slots.
                            deferred_fin[0]()
                            deferred_fin[0] = None
                        heads = (2 * pair, 2 * pair + 1)
                        sums_of = {heads[0]: sums_a, heads[1]: sums_b}
                        ovs = {}
                        for h in heads:
                            ov = ps2.tile(
                                [DK + 1, 512], f32, tag="ovpo", bufs=2, name=f"ov{h}q{qb}"
                            )
                            ovs[h] = ov
                        # chunked kb loop with one-chunk lag: emit a
                        # chunk of scores+exps, then the PREVIOUS chunk's
                        # attnV matmuls grouped per head (consecutive
                        # same-bank accumulation -- interleaving score and
                        # attnV matmuls forces a PE pipeline drain at every
                        # lhsT row-group conflict, measured 733 vs 237 ns/mm)
                        def emit_avs(items):
                            grp = []
                            for h in heads:
                                for (pkb, pcs, pets) in items:
                                    grp.append(nc.tensor.matmul(
                                        ovs[h][:, pcs:], V[:, pkb, h, :],
                                        pets[h][:, pcs:],
                                        start=(pkb == 0), stop=(pkb == nkb - 1),
                                    ))
                            pe_group(grp)

                        CH = 3
                        kbs = list(range(nkb))
                        chunks = [kbs[i : i + CH] for i in range(0, nkb, CH)]
                        pend = None
                        for chunk in chunks:
                            items = []
                            sc_grp = []
                            for kb in chunk:
                                ksl = slice(kb * 128, (kb + 1) * 128)
                                d = kb - 4 * qb
                                cs = 128 * d if d > 0 else 0
                                ets = {}
                                for h in heads:
                                    base = 64 * (h % 2)
                                    psl = slice(base, base + 64)
                                    sp = ps2.tile(
                                        [128, 512], f32, tag="score", bufs=6,
                                        name=f"sp{h}q{qb}k{kb}",
                                    )
                                    sc_grp.append(nc.tensor.matmul(
                                        sp, KT[psl, pair, ksl], QT[psl, pair, qsl],
                                        start=True, stop=True,
                                    ))
                                    if d >= 0:
                                        nc.vector.tensor_tensor(
                                            sp[:, cs : cs + 128],
                                            sp[:, cs : cs + 128],
                                            maskb[:, :],
                                            ADD,
                                        )
                                    et = s2p.tile(
                                        [128, 512], f32r, tag="et", bufs=12,
                                        name=f"et{h}q{qb}k{kb}",
                                    )
                                    nc.scalar.activation(
                                        et[:, cs:], sp[:, cs:], AF.Exp, scale=0.125
                                    )
                                    ets[h] = et
                                items.append((kb, cs, ets))
                            pe_group(sc_grp)
                            if pend is not None:
                                emit_avs(pend)
                            pend = items
                        emit_avs(pend)
                        for h in heads:
                            base = 64 * (h % 2)
                            ov = ovs[h]
                            # gather denominator row; write unnormalized AT
                            nc.vector.tensor_copy(
                                sums_of[h][pair * 32 : pair * 32 + 1, :],
                                ov[DK : DK + 1, :],
                            )
                            nc.any.tensor_copy(
                                AT[base : base + 64, pair, qsl], ov[0:DK, :]
                            )
                    # normalize all 8 heads of this q-block: batched
                    # reciprocal, then per head a K=1 broadcast matmul and an
                    # in-place multiply on AT.
                    srec_a = s2p.tile([128, 512], f32r, tag="srec_a", bufs=2, name=f"srec_a{qb}")
                    srec_b = s2p.tile([128, 512], f32r, tag="srec_b", bufs=2, name=f"srec_b{qb}")
                    with nc.allow_low_precision(reason="f32r is fp32-width"):
                        nc.vector.reciprocal(srec_a[:], sums_a[:])
                        nc.vector.reciprocal(srec_b[:], sums_b[:])
                    def finalize(qb=qb, qsl=qsl, srec_a=srec_a, srec_b=srec_b):
                        rbs_ = {}
                        rb_grp = []
                        for pair2 in range(4):
                            heads2 = (2 * pair2, 2 * pair2 + 1)
                            srec_of = {heads2[0]: srec_a, heads2[1]: srec_b}
                            for h in heads2:
                                p32 = pair2 * 32
                                rb = ps2.tile([128, 512], f32, tag="score", bufs=6, name=f"rb{h}{qb}")
                                rb_grp.append(nc.tensor.matmul(
                                    rb[0:64, :],
                                    ones_r[p32 : p32 + 1, :],
                                    srec_of[h][p32 : p32 + 1, :],
                                    start=True, stop=True,
                                    tile_position=(p32, 0) if p32 == 96 else None,
                                ))
                                rbs_[h] = rb
                        pe_group(rb_grp)
                        for pair2 in range(4):
                            for h in (2 * pair2, 2 * pair2 + 1):
                                base = 64 * (h % 2)
                                nc.vector.tensor_tensor(
                                    AT[base : base + 64, pair2, qsl],
                                    AT[base : base + 64, pair2, qsl].bitcast(f32),
                                    rbs_[h][0:64, :],
                                    MULT,
                                )
                        for sc in range(4 * qb, 4 * qb + 4):
                            og = s2p.tile([128, D], f32, tag="og", bufs=2, name=f"og{sc}")
                            po_grp = []
                            for jh in range(2):
                                po = ps2.tile(
                                    [128, 512], f32, tag="ovpo", bufs=2, name=f"po{sc}{jh}"
                                )
                                for io in range(4):
                                    po_grp.append(nc.tensor.matmul(
                                        po,
                                        AT[:, io, sc * 128 : (sc + 1) * 128],
                                        wo[:, io, jh * 512 : (jh + 1) * 512],
                                        start=(io == 0), stop=(io == 3),
                                    ))
                                nc.any.tensor_copy(og[:, jh * 512 : (jh + 1) * 512], po[:])
                            pe_group(po_grp)
                            nc.sync.dma_start(out_d[sc * 128 : (sc + 1) * 128, :], og[:])

                    deferred_fin[0] = finalize
                if deferred_fin[0] is not None:
                    deferred_fin[0]()

    nc.compile()
    _BUILD_CACHE["nc"] = nc
    return nc


def _host_inputs(x, WQ, WK, WV, WO):
    ki = np.arange(128, dtype=np.float32)[:, None]
    qj = np.arange(128, dtype=np.float32)[None, :]
    # stripe mask: within the partial 128-col stripe of diagonal chunk d,
    # allowed iff (qj - 128d) >= ki, i.e. local column >= ki.
    mask = np.where(qj >= ki, 0.0, NEG).astype(np.float32)
    onesb = np.ones((128, 128), dtype=np.float32)

    in_maps = []
    for b in range(B):
        xT = np.ascontiguousarray(x[b].T)
        for g in range(G):
            sl = slice(g * DG, (g + 1) * DG)
            in_maps.append(
                {
                    "xT": xT,
                    "wqT": np.ascontiguousarray(WQ[sl, :].T),
                    "wkT": np.ascontiguousarray(WK[sl, :].T),
                    "wvT": np.ascontiguousarray(WV[sl, :].T),
                    "woT": np.ascontiguousarray(WO[:, sl].T),
                    "mask": mask,
                    "onesb": onesb,
                }
            )
    return in_maps


def kernel(x, WQ, WK, WV, WO):
    from concourse.bass_utils import run_bass_kernel_spmd

    x = np.asarray(x, dtype=np.float32)
    WQ = np.asarray(WQ, dtype=np.float32)
    WK = np.asarray(WK, dtype=np.float32)
    WV = np.asarray(WV, dtype=np.float32)
    WO = np.asarray(WO, dtype=np.float32)

    nc = _build()
    in_maps = _host_inputs(x, WQ, WK, WV, WO)
    res = run_bass_kernel_spmd(
        nc,
        in_maps,
        core_ids=list(range(8)),
        trace=bool(os.environ.get("KERNEL_TRACE")),
    )
    kernel.last_results = res
    parts = [r["out"] for r in res.results]
    out = np.stack([parts[2 * b] + parts[2 * b + 1] for b in range(B)], axis=0)
    return out.astype(np.float32)


# revision 13
# speedup vs baseline: 1.4880x; 1.0009x over previous
"""Causal multi-head self-attention on 8 Trainium2 NeuronCores.

Sharding: core = (batch b, head-group g).  B=4 batches x 2 groups of 8 heads
= 8 cores.  Each core computes Q/K/V projections for its 8 heads, causal
attention, and a partial output projection (row-shard of WO); the host sums
the two partials per batch (the tensor-parallel all-reduce, done at gather).

Per-core device pipeline (all matmuls in float32r = full fp32 precision via
the PE's 2-pass mode, 1 cycle/row at N>=256):
  stage 1: QT[d',s], KT[d',s] (transposed) and V[s,d'] (natural) projections
           from host-pre-transposed xT and weight shards.
  stage 2: q-block outer loop; per (q-block, head-pair): scoresT[k,q] per
           128-wide k-chunk, causal mask add on the single partial 128-col
           stripe of diagonal chunks, exp over the allowed column range only
           (no max subtraction -- scores are O(5) so exp is safe in fp32),
           attn@V with a ones column appended to V so PSUM row 64
           accumulates the softmax denominator.  The kb loop is software
           pipelined (attnV lags scores by one chunk) so the in-order PE
           stream never stalls on the ACT exp.  Denominators gather into
           [128, 512] tiles (rows at pair*32) so reciprocals run 8 rows at
           a time; normalization is an in-place multiply on AT.
  stage 3: output projection for the finished q rows, interleaved with the
           next q-block's attention.
"""

import os
import numpy as np

B, S, D = 4, 2048, 1024
H_TOTAL, DK = 16, 64
G = 2          # head groups (cores per batch)
HG = 8         # heads per core
DG = 512       # head dims per core
CO = 8         # contraction chunks of 128 over D
SBLK = 4       # 512-wide s blocks
QB = 4         # 512-wide q blocks
NEG = -1e9

_BUILD_CACHE = {}


def _build():
    if "nc" in _BUILD_CACHE:
        return _BUILD_CACHE["nc"]

    import concourse.bacc as bacc
    import concourse.mybir as mybir
    import concourse.tile as tile
    from concourse.tile_rust import add_dep_helper

    f32 = mybir.dt.float32
    f32r = mybir.dt.float32r
    AF = mybir.ActivationFunctionType
    ADD = mybir.AluOpType.add
    MULT = mybir.AluOpType.mult

    nc = bacc.Bacc("TRN2", target_bir_lowering=False)
    xT_d = nc.dram_tensor("xT", [D, S], f32, kind="ExternalInput")
    wq_d = nc.dram_tensor("wqT", [D, DG], f32, kind="ExternalInput")
    wk_d = nc.dram_tensor("wkT", [D, DG], f32, kind="ExternalInput")
    wv_d = nc.dram_tensor("wvT", [D, DG], f32, kind="ExternalInput")
    wo_d = nc.dram_tensor("woT", [DG, D], f32, kind="ExternalInput")
    mask_d = nc.dram_tensor("mask", [128, 128], f32, kind="ExternalInput")
    ones_d = nc.dram_tensor("onesb", [128, 128], f32, kind="ExternalInput")
    out_d = nc.dram_tensor("out", [S, D], f32, kind="ExternalOutput")

    with tile.TileContext(nc) as tc:
        with tc.tile_pool(name="persist", bufs=1) as pp:
            QT = pp.tile([128, 4, S], f32r, tag="QT")
            KT = pp.tile([128, 4, S], f32r, tag="KT")
            V = pp.tile([128, 16, HG, DK + 1], f32r, tag="V")
            maskb = pp.tile([128, 128], f32, tag="maskb")
            onesb = pp.tile([128, 128], f32, tag="onesb")
            ones_r = pp.tile([128, 64], f32r, tag="ones_r")
            nc.sync.dma_start(maskb[:], mask_d[:, :])
            nc.sync.dma_start(onesb[:], ones_d[:, :])
            nc.sync.dma_start(ones_r[:], ones_d[:, 0:64].bitcast(f32r))
            # ones column of V (f32 -> f32r rounding copy)
            nc.vector.tensor_copy(
                V[:, :, :, DK : DK + 1],
                onesb[:, 0:128].rearrange("p (so h) -> p so h", so=16)[:, :, :, None],
            )

            # ---------------- stage 1: projections ----------------
            with (
                tc.tile_pool(name="stage1", bufs=1) as s1p,
                tc.tile_pool(name="ps1", bufs=1, space="PSUM") as ps1,
            ):
                wq = s1p.tile([128, CO, DG], f32r, tag="wq")
                wk = s1p.tile([128, CO, DG], f32r, tag="wk")
                wv = s1p.tile([128, CO, DG], f32r, tag="wv")
                nc.sync.dma_start(
                    wq, wq_d[:, :].rearrange("(co ci) d -> ci co d", ci=128).bitcast(f32r)
                )
                nc.sync.dma_start(
                    wk, wk_d[:, :].rearrange("(co ci) d -> ci co d", ci=128).bitcast(f32r)
                )
                nc.sync.dma_start(
                    wv, wv_d[:, :].rearrange("(co ci) d -> ci co d", ci=128).bitcast(f32r)
                )
                for sb in range(SBLK):
                    xt = s1p.tile([128, CO, 512], f32r, tag="xt", bufs=2)
                    nc.sync.dma_start(
                        xt,
                        xT_d[:, sb * 512 : (sb + 1) * 512]
                        .rearrange("(co ci) s -> ci co s", ci=128)
                        .bitcast(f32r),
                    )
                    ssl = slice(sb * 512, (sb + 1) * 512)
                    for do in range(4):
                        dsl = slice(do * 128, (do + 1) * 128)
                        pq = ps1.tile([128, 512], f32, tag="proj", bufs=4, name=f"pq{sb}{do}")
                        for co in range(CO):
                            nc.tensor.matmul(
                                pq, wq[:, co, dsl], xt[:, co, :],
                                start=(co == 0), stop=(co == CO - 1),
                            )
                        nc.any.tensor_copy(QT[:, do, ssl], pq[:])
                        pk = ps1.tile([128, 512], f32, tag="proj", bufs=4, name=f"pk{sb}{do}")
                        for co in range(CO):
                            nc.tensor.matmul(
                                pk, wk[:, co, dsl], xt[:, co, :],
                                start=(co == 0), stop=(co == CO - 1),
                            )
                        nc.any.tensor_copy(KT[:, do, ssl], pk[:])
                    for so in range(4):
                        pv = ps1.tile([128, 512], f32, tag="proj", bufs=4, name=f"pv{sb}{so}")
                        for co in range(CO):
                            nc.tensor.matmul(
                                pv, xt[:, co, so * 128 : (so + 1) * 128], wv[:, co, :],
                                start=(co == 0), stop=(co == CO - 1),
                            )
                        nc.any.tensor_copy(
                            V[:, sb * 4 + so, :, 0:DK],
                            pv[:].rearrange("p (h d) -> p h d", h=HG),
                        )

            # ---------------- stages 2+3: attention + output ----------------
            with (
                tc.tile_pool(name="atp", bufs=1) as atp,
                tc.tile_pool(name="stage2", bufs=1) as s2p,
                tc.tile_pool(name="ps2", bufs=1, space="PSUM") as ps2,
            ):
                AT = atp.tile([128, 4, S], f32r, tag="AT")
                wo = s2p.tile([128, 4, D], f32r, tag="wo")
                nc.sync.dma_start(
                    wo,
                    wo_d[:, :].rearrange("(io ip) j -> ip io j", ip=128).bitcast(f32r),
                )
                pe_prev = [None]  # last instr of the previous PE group

                def pe_group(insts):
                    # force PE issue order: first of this group after last of
                    # the previous group; chain within the group
                    if not insts:
                        return
                    if pe_prev[0] is not None:
                        add_dep_helper(
                            insts[0].ins, pe_prev[0].ins, sync=False,
                            reason="pe group order",
                        )
                    for a, b in zip(insts[1:], insts):
                        add_dep_helper(a.ins, b.ins, sync=False, reason="pe chain")
                    pe_prev[0] = insts[-1]

                deferred_fin = [None]

                for qb in range(QB):
                    qsl = slice(qb * 512, (qb + 1) * 512)
                    nkb = 4 * qb + 4
                    # denominator gather tiles; rows at pair*32 (SBUF
                    # partition offsets must be 32-aligned).  memset to 1.0
                    # so the batched reciprocal sees no garbage lanes.
                    sums_a = s2p.tile([128, 512], f32, tag="sums_a", bufs=2, name=f"sums_a{qb}")
                    sums_b = s2p.tile([128, 512], f32, tag="sums_b", bufs=2, name=f"sums_b{qb}")
                    nc.gpsimd.memset(sums_a[:], 1.0)
                    nc.gpsimd.memset(sums_b[:], 1.0)
                    for pair in range(4):
                        if pair == 1 and deferred_fin[0] is not None:
                            # fire the previous q-block's normalization +
                            # output projection here: pair 0's ov slots are
                            # released and pair 1's not yet allocated, so the
                            # deferred po tiles take clean ovpo ring slots.
                            deferred_fin[0]()
                            deferred_fin[0] = None
                        heads = (2 * pair, 2 * pair + 1)
                        sums_of = {heads[0]: sums_a, heads[1]: sums_b}
                        ovs = {}
                        for h in heads:
                            ov = ps2.tile(
                                [DK + 1, 512], f32, tag="ovpo", bufs=2, name=f"ov{h}q{qb}"
                            )
                            ovs[h] = ov
                        # chunked kb loop with one-chunk lag: emit a
                        # chunk of scores+exps, then the PREVIOUS chunk's
                        # attnV matmuls grouped per head (consecutive
                        # same-bank accumulation -- interleaving score and
                        # attnV matmuls forces a PE pipeline drain at every
                        # lhsT row-group conflict, measured 733 vs 237 ns/mm)
                        def emit_avs(items):
                            grp = []
                            for h in heads:
                                for (pkb, pcs, pets) in items:
                                    grp.append(nc.tensor.matmul(
                                        ovs[h][:, pcs:], V[:, pkb, h, :],
                                        pets[h][:, pcs:],
                                        start=(pkb == 0), stop=(pkb == nkb - 1),
                                    ))
                            pe_group(grp)

                        CH = 3
                        kbs = list(range(nkb))
                        chunks = [kbs[i : i + CH] for i in range(0, nkb, CH)]
                        pend = None
                        for chunk in chunks:
                            items = []
                            sc_grp = []
                            for kb in chunk:
                                ksl = slice(kb * 128, (kb + 1) * 128)
                                d = kb - 4 * qb
                                cs = 128 * d if d > 0 else 0
                                ets = {}
                                for h in heads:
                                    base = 64 * (h % 2)
                                    psl = slice(base, base + 64)
                                    sp = ps2.tile(
                                        [128, 512], f32, tag="score", bufs=6,
                                        name=f"sp{h}q{qb}k{kb}",
                                    )
                                    sc_grp.append(nc.tensor.matmul(
                                        sp, KT[psl, pair, ksl], QT[psl, pair, qsl],
                                        start=True, stop=True,
                                    ))
                                    if d >= 0:
                                        nc.vector.tensor_tensor(
                                            sp[:, cs : cs + 128],
                                            sp[:, cs : cs + 128],
                                            maskb[:, :],
                                            ADD,
                                        )
                                    et = s2p.tile(
                                        [128, 512], f32r, tag="et", bufs=12,
                                        name=f"et{h}q{qb}k{kb}",
                                    )
                                    nc.scalar.activation(
                                        et[:, cs:], sp[:, cs:], AF.Exp, scale=0.125
                                    )
                                    ets[h] = et
                                items.append((kb, cs, ets))
                            pe_group(sc_grp)
                            if pend is not None:
                                emit_avs(pend)
                            pend = items
                        emit_avs(pend)
                        for h in heads:
                            base = 64 * (h % 2)
                            ov = ovs[h]
                            # gather denominator row; write unnormalized AT
                            nc.vector.tensor_copy(
                                sums_of[h][pair * 32 : pair * 32 + 1, :],
                                ov[DK : DK + 1, :],
                            )
                            nc.any.tensor_copy(
                                AT[base : base + 64, pair, qsl], ov[0:DK, :]
                            )
                    # normalize all 8 heads of this q-block: batched
                    # reciprocal, then per head a K=1 broadcast matmul and an
                    # in-place multiply on AT.
                    srec_a = s2p.tile([128, 512], f32r, tag="srec_a", bufs=2, name=f"srec_a{qb}")
                    srec_b = s2p.tile([128, 512], f32r, tag="srec_b", bufs=2, name=f"srec_b{qb}")
                    with nc.allow_low_precision(reason="f32r is fp32-width"):
                        nc.vector.reciprocal(srec_a[:], sums_a[:])
                        nc.vector.reciprocal(srec_b[:], sums_b[:])
                    def finalize(qb=qb, qsl=qsl, srec_a=srec_a, srec_b=srec_b):
                        rbs_ = {}
                        rb_grp = []
                        for pair2 in range(4):
                            heads2 = (2 * pair2, 2 * pair2 + 1)
                            srec_of = {heads2[0]: srec_a, heads2[1]: srec_b}
                            for h in heads2:
                                p32 = pair2 * 32
                                rb = ps2.tile([128, 512], f32, tag="score", bufs=6, name=f"rb{h}{qb}")
                                rb_grp.append(nc.tensor.matmul(
                                    rb[0:64, :],
                                    ones_r[p32 : p32 + 1, :],
                                    srec_of[h][p32 : p32 + 1, :],
                                    start=True, stop=True,
                                    tile_position=(p32, 0) if p32 == 96 else None,
                                ))
                                rbs_[h] = rb
                        pe_group(rb_grp)
                        for pair2 in range(4):
                            for h in (2 * pair2, 2 * pair2 + 1):
                                base = 64 * (h % 2)
                                nc.any.tensor_tensor(
                                    AT[base : base + 64, pair2, qsl],
                                    AT[base : base + 64, pair2, qsl].bitcast(f32),
                                    rbs_[h][0:64, :],
                                    MULT,
                                )
                        for sc in range(4 * qb, 4 * qb + 4):
                            og = s2p.tile([128, D], f32, tag="og", bufs=2, name=f"og{sc}")
                            po_grp = []
                            for jh in range(2):
                                po = ps2.tile(
                                    [128, 512], f32, tag="ovpo", bufs=2, name=f"po{sc}{jh}"
                                )
                                for io in range(4):
                                    po_grp.append(nc.tensor.matmul(
                                        po,
                                        AT[:, io, sc * 128 : (sc + 1) * 128],
                                        wo[:, io, jh * 512 : (jh + 1) * 512],
                                        start=(io == 0), stop=(io == 3),
                                    ))
                                nc.any.tensor_copy(og[:, jh * 512 : (jh + 1) * 512], po[:])
                            pe_group(po_grp)
                            nc.sync.dma_start(out_d[sc * 128 : (sc + 1) * 128, :], og[:])

                    deferred_fin[0] = finalize
                if deferred_fin[0] is not None:
                    deferred_fin[0]()

    nc.compile()
    _BUILD_CACHE["nc"] = nc
    return nc


def _host_inputs(x, WQ, WK, WV, WO):
    ki = np.arange(128, dtype=np.float32)[:, None]
    qj = np.arange(128, dtype=np.float32)[None, :]
    # stripe mask: within the partial 128-col stripe of diagonal chunk d,
    # allowed iff (qj - 128d) >= ki, i.e. local column >= ki.
    mask = np.where(qj >= ki, 0.0, NEG).astype(np.float32)
    onesb = np.ones((128, 128), dtype=np.float32)

    in_maps = []
    for b in range(B):
        xT = np.ascontiguousarray(x[b].T)
        for g in range(G):
            sl = slice(g * DG, (g + 1) * DG)
            in_maps.append(
                {
                    "xT": xT,
                    "wqT": np.ascontiguousarray(WQ[sl, :].T),
                    "wkT": np.ascontiguousarray(WK[sl, :].T),
                    "wvT": np.ascontiguousarray(WV[sl, :].T),
                    "woT": np.ascontiguousarray(WO[:, sl].T),
                    "mask": mask,
                    "onesb": onesb,
                }
            )
    return in_maps


def kernel(x, WQ, WK, WV, WO):
    from concourse.bass_utils import run_bass_kernel_spmd

    x = np.asarray(x, dtype=np.float32)
    WQ = np.asarray(WQ, dtype=np.float32)
    WK = np.asarray(WK, dtype=np.float32)
    WV = np.asarray(WV, dtype=np.float32)
    WO = np.asarray(WO, dtype=np.float32)

    nc = _build()
    in_maps = _host_inputs(x, WQ, WK, WV, WO)
    res = run_bass_kernel_spmd(
        nc,
        in_maps,
        core_ids=list(range(8)),
        trace=bool(os.environ.get("KERNEL_TRACE")),
    )
    kernel.last_results = res
    parts = [r["out"] for r in res.results]
    out = np.stack([parts[2 * b] + parts[2 * b + 1] for b in range(B)], axis=0)
    return out.astype(np.float32)


# revision 14
# speedup vs baseline: 1.4968x; 1.0059x over previous
"""Causal multi-head self-attention on 8 Trainium2 NeuronCores.

Sharding: core = (batch b, head-group g).  B=4 batches x 2 groups of 8 heads
= 8 cores.  Each core computes Q/K/V projections for its 8 heads, causal
attention, and a partial output projection (row-shard of WO); the host sums
the two partials per batch (the tensor-parallel all-reduce, done at gather).

Per-core device pipeline (all matmuls in float32r = full fp32 precision via
the PE's 2-pass mode, 1 cycle/row at N>=256):
  stage 1: QT[d',s], KT[d',s] (transposed) and V[s,d'] (natural) projections
           from host-pre-transposed xT and weight shards.
  stage 2: q-block outer loop; per (q-block, head-pair): scoresT[k,q] per
           128-wide k-chunk, causal mask add on the single partial 128-col
           stripe of diagonal chunks, exp over the allowed column range only
           (no max subtraction -- scores are O(5) so exp is safe in fp32),
           attn@V with a ones column appended to V so PSUM row 64
           accumulates the softmax denominator.  The kb loop is software
           pipelined (attnV lags scores by one chunk) so the in-order PE
           stream never stalls on the ACT exp.  Denominators gather into
           [128, 512] tiles (rows at pair*32) so reciprocals run 8 rows at
           a time; normalization is an in-place multiply on AT.
  stage 3: output projection for the finished q rows, interleaved with the
           next q-block's attention.
"""

import os
import numpy as np

B, S, D = 4, 2048, 1024
H_TOTAL, DK = 16, 64
G = 2          # head groups (cores per batch)
HG = 8         # heads per core
DG = 512       # head dims per core
CO = 8         # contraction chunks of 128 over D
SBLK = 4       # 512-wide s blocks
QB = 4         # 512-wide q blocks
NEG = -1e9

_BUILD_CACHE = {}


def _build():
    if "nc" in _BUILD_CACHE:
        return _BUILD_CACHE["nc"]

    import concourse.bacc as bacc
    import concourse.mybir as mybir
    import concourse.tile as tile
    from concourse.tile_rust import add_dep_helper

    f32 = mybir.dt.float32
    f32r = mybir.dt.float32r
    AF = mybir.ActivationFunctionType
    ADD = mybir.AluOpType.add
    MULT = mybir.AluOpType.mult

    nc = bacc.Bacc("TRN2", target_bir_lowering=False)
    xT_d = nc.dram_tensor("xT", [D, S], f32, kind="ExternalInput")
    wq_d = nc.dram_tensor("wqT", [D, DG], f32, kind="ExternalInput")
    wk_d = nc.dram_tensor("wkT", [D, DG], f32, kind="ExternalInput")
    wv_d = nc.dram_tensor("wvT", [D, DG], f32, kind="ExternalInput")
    wo_d = nc.dram_tensor("woT", [DG, D], f32, kind="ExternalInput")
    mask_d = nc.dram_tensor("mask", [128, 128], f32, kind="ExternalInput")
    ones_d = nc.dram_tensor("onesb", [128, 128], f32, kind="ExternalInput")
    out_d = nc.dram_tensor("out", [S, D], f32, kind="ExternalOutput")

    with tile.TileContext(nc) as tc:
        with tc.tile_pool(name="persist", bufs=1) as pp:
            QT = pp.tile([128, 4, S], f32r, tag="QT")
            KT = pp.tile([128, 4, S], f32r, tag="KT")
            V = pp.tile([128, 16, HG, DK + 1], f32r, tag="V")
            maskb = pp.tile([128, 128], f32, tag="maskb")
            onesb = pp.tile([128, 128], f32, tag="onesb")
            ones_r = pp.tile([128, 64], f32r, tag="ones_r")
            nc.sync.dma_start(maskb[:], mask_d[:, :])
            nc.sync.dma_start(onesb[:], ones_d[:, :])
            nc.sync.dma_start(ones_r[:], ones_d[:, 0:64].bitcast(f32r))
            # ones column of V (f32 -> f32r rounding copy)
            nc.vector.tensor_copy(
                V[:, :, :, DK : DK + 1],
                onesb[:, 0:128].rearrange("p (so h) -> p so h", so=16)[:, :, :, None],
            )

            # ---------------- stage 1: projections ----------------
            with (
                tc.tile_pool(name="stage1", bufs=1) as s1p,
                tc.tile_pool(name="ps1", bufs=1, space="PSUM") as ps1,
            ):
                wq = s1p.tile([128, CO, DG], f32r, tag="wq")
                wk = s1p.tile([128, CO, DG], f32r, tag="wk")
                wv = s1p.tile([128, CO, DG], f32r, tag="wv")
                nc.sync.dma_start(
                    wq, wq_d[:, :].rearrange("(co ci) d -> ci co d", ci=128).bitcast(f32r)
                )
                nc.sync.dma_start(
                    wk, wk_d[:, :].rearrange("(co ci) d -> ci co d", ci=128).bitcast(f32r)
                )
                nc.sync.dma_start(
                    wv, wv_d[:, :].rearrange("(co ci) d -> ci co d", ci=128).bitcast(f32r)
                )
                for sb in range(SBLK):
                    xt = s1p.tile([128, CO, 512], f32r, tag="xt", bufs=2)
                    nc.sync.dma_start(
                        xt,
                        xT_d[:, sb * 512 : (sb + 1) * 512]
                        .rearrange("(co ci) s -> ci co s", ci=128)
                        .bitcast(f32r),
                    )
                    ssl = slice(sb * 512, (sb + 1) * 512)
                    for do in range(4):
                        dsl = slice(do * 128, (do + 1) * 128)
                        pq = ps1.tile([128, 512], f32, tag="proj", bufs=4, name=f"pq{sb}{do}")
                        for co in range(CO):
                            nc.tensor.matmul(
                                pq, wq[:, co, dsl], xt[:, co, :],
                                start=(co == 0), stop=(co == CO - 1),
                            )
                        nc.any.tensor_copy(QT[:, do, ssl], pq[:])
                        pk = ps1.tile([128, 512], f32, tag="proj", bufs=4, name=f"pk{sb}{do}")
                        for co in range(CO):
                            nc.tensor.matmul(
                                pk, wk[:, co, dsl], xt[:, co, :],
                                start=(co == 0), stop=(co == CO - 1),
                            )
                        nc.any.tensor_copy(KT[:, do, ssl], pk[:])
                    for so in range(4):
                        pv = ps1.tile([128, 512], f32, tag="proj", bufs=4, name=f"pv{sb}{so}")
                        for co in range(CO):
                            nc.tensor.matmul(
                                pv, xt[:, co, so * 128 : (so + 1) * 128], wv[:, co, :],
                                start=(co == 0), stop=(co == CO - 1),
                            )
                        nc.any.tensor_copy(
                            V[:, sb * 4 + so, :, 0:DK],
                            pv[:].rearrange("p (h d) -> p h d", h=HG),
                        )

            # ---------------- stages 2+3: attention + output ----------------
            with (
                tc.tile_pool(name="atp", bufs=1) as atp,
                tc.tile_pool(name="stage2", bufs=1) as s2p,
                tc.tile_pool(name="ps2", bufs=1, space="PSUM") as ps2,
            ):
                AT = atp.tile([128, 4, S], f32r, tag="AT")
                wo = s2p.tile([128, 4, D], f32r, tag="wo")
                nc.sync.dma_start(
                    wo,
                    wo_d[:, :].rearrange("(io ip) j -> ip io j", ip=128).bitcast(f32r),
                )
                pe_prev = [None]  # last instr of the previous PE group

                def pe_group(insts):
                    # force PE issue order: first of this group after last of
                    # the previous group; chain within the group
                    if not insts:
                        return
                    if pe_prev[0] is not None:
                        add_dep_helper(
                            insts[0].ins, pe_prev[0].ins, sync=False,
                            reason="pe group order",
                        )
                    for a, b in zip(insts[1:], insts):
                        add_dep_helper(a.ins, b.ins, sync=False, reason="pe chain")
                    pe_prev[0] = insts[-1]

                deferred_fin = [None]

                for qb in range(QB):
                    qsl = slice(qb * 512, (qb + 1) * 512)
                    nkb = 4 * qb + 4
                    # denominator gather tiles; rows at pair*32 (SBUF
                    # partition offsets must be 32-aligned).  memset to 1.0
                    # so the batched reciprocal sees no garbage lanes.
                    sums_a = s2p.tile([128, 512], f32, tag="sums_a", bufs=2, name=f"sums_a{qb}")
                    sums_b = s2p.tile([128, 512], f32, tag="sums_b", bufs=2, name=f"sums_b{qb}")
                    nc.gpsimd.memset(sums_a[:], 1.0)
                    nc.gpsimd.memset(sums_b[:], 1.0)
                    for pair in range(4):
                        if pair == 1 and deferred_fin[0] is not None:
                            # fire the previous q-block's normalization +
                            # output projection here: pair 0's ov slots are
                            # released and pair 1's not yet allocated, so the
                            # deferred po tiles take clean ovpo ring slots.
                            deferred_fin[0]()
                            deferred_fin[0] = None
                        heads = (2 * pair, 2 * pair + 1)
                        sums_of = {heads[0]: sums_a, heads[1]: sums_b}
                        ovs = {}
                        for h in heads:
                            ov = ps2.tile(
                                [DK + 1, 512], f32, tag="ovpo", bufs=2, name=f"ov{h}q{qb}"
                            )
                            ovs[h] = ov
                        # chunked kb loop with one-chunk lag: emit a
                        # chunk of scores+exps, then the PREVIOUS chunk's
                        # attnV matmuls grouped per head (consecutive
                        # same-bank accumulation -- interleaving score and
                        # attnV matmuls forces a PE pipeline drain at every
                        # lhsT row-group conflict, measured 733 vs 237 ns/mm)
                        def emit_avs(items):
                            grp = []
                            for h in heads:
                                for (pkb, pcs, pets) in items:
                                    grp.append(nc.tensor.matmul(
                                        ovs[h][:, pcs:], V[:, pkb, h, :],
                                        pets[h][:, pcs:],
                                        start=(pkb == 0), stop=(pkb == nkb - 1),
                                    ))
                            pe_group(grp)

                        CH = 3
                        kbs = list(range(nkb))
                        chunks = [kbs[i : i + CH] for i in range(0, nkb, CH)]
                        pend = None
                        for chunk in chunks:
                            items = []
                            sc_grp = []
                            for kb in chunk:
                                ksl = slice(kb * 128, (kb + 1) * 128)
                                d = kb - 4 * qb
                                cs = 128 * d if d > 0 else 0
                                ets = {}
                                for h in heads:
                                    base = 64 * (h % 2)
                                    psl = slice(base, base + 64)
                                    sp = ps2.tile(
                                        [128, 512], f32, tag="score", bufs=6,
                                        name=f"sp{h}q{qb}k{kb}",
                                    )
                                    sc_grp.append(nc.tensor.matmul(
                                        sp, KT[psl, pair, ksl], QT[psl, pair, qsl],
                                        start=True, stop=True,
                                    ))
                                    if d >= 0:
                                        nc.vector.tensor_tensor(
                                            sp[:, cs : cs + 128],
                                            sp[:, cs : cs + 128],
                                            maskb[:, :],
                                            ADD,
                                        )
                                    et = s2p.tile(
                                        [128, 512], f32r, tag="et", bufs=12,
                                        name=f"et{h}q{qb}k{kb}",
                                    )
                                    nc.scalar.activation(
                                        et[:, cs:], sp[:, cs:], AF.Exp, scale=0.125
                                    )
                                    ets[h] = et
                                items.append((kb, cs, ets))
                            pe_group(sc_grp)
                            if pend is not None:
                                emit_avs(pend)
                            pend = items
                        emit_avs(pend)
                        for h in heads:
                            base = 64 * (h % 2)
                            ov = ovs[h]
                            # gather denominator row; write unnormalized AT
                            nc.vector.tensor_copy(
                                sums_of[h][pair * 32 : pair * 32 + 1, :],
                                ov[DK : DK + 1, :],
                            )
                            nc.vector.tensor_copy(
                                AT[base : base + 64, pair, qsl], ov[0:DK, :]
                            )
                    # normalize all 8 heads of this q-block: batched
                    # reciprocal, then per head a K=1 broadcast matmul and an
                    # in-place multiply on AT.
                    srec_a = s2p.tile([128, 512], f32r, tag="srec_a", bufs=2, name=f"srec_a{qb}")
                    srec_b = s2p.tile([128, 512], f32r, tag="srec_b", bufs=2, name=f"srec_b{qb}")
                    with nc.allow_low_precision(reason="f32r is fp32-width"):
                        nc.vector.reciprocal(srec_a[:], sums_a[:])
                        nc.vector.reciprocal(srec_b[:], sums_b[:])
                    def finalize(qb=qb, qsl=qsl, srec_a=srec_a, srec_b=srec_b):
                        rbs_ = {}
                        rb_grp = []
                        for pair2 in range(4):
                            heads2 = (2 * pair2, 2 * pair2 + 1)
                            srec_of = {heads2[0]: srec_a, heads2[1]: srec_b}
                            for h in heads2:
                                p32 = pair2 * 32
                                rb = ps2.tile([128, 512], f32, tag="score", bufs=6, name=f"rb{h}{qb}")
                                rb_grp.append(nc.tensor.matmul(
                                    rb[0:64, :],
                                    ones_r[p32 : p32 + 1, :],
                                    srec_of[h][p32 : p32 + 1, :],
                                    start=True, stop=True,
                                    tile_position=(p32, 0) if p32 == 96 else None,
                                ))
                                rbs_[h] = rb
                        pe_group(rb_grp)
                        for pair2 in range(4):
                            for h in (2 * pair2, 2 * pair2 + 1):
                                base = 64 * (h % 2)
                                nc.any.tensor_tensor(
                                    AT[base : base + 64, pair2, qsl],
                                    AT[base : base + 64, pair2, qsl].bitcast(f32),
                                    rbs_[h][0:64, :],
                                    MULT,
                                )
                        for sc in range(4 * qb, 4 * qb + 4):
                            og = s2p.tile([128, D], f32, tag="og", bufs=2, name=f"og{sc}")
                            po_grp = []
                            for jh in range(2):
                                po = ps2.tile(
                                    [128, 512], f32, tag="ovpo", bufs=2, name=f"po{sc}{jh}"
                                )
                                for io in range(4):
                                    po_grp.append(nc.tensor.matmul(
                                        po,
                                        AT[:, io, sc * 128 : (sc + 1) * 128],
                                        wo[:, io, jh * 512 : (jh + 1) * 512],
                                        start=(io == 0), stop=(io == 3),
                                    ))
                                nc.vector.tensor_copy(og[:, jh * 512 : (jh + 1) * 512], po[:])
                            pe_group(po_grp)
                            nc.sync.dma_start(out_d[sc * 128 : (sc + 1) * 128, :], og[:])

                    deferred_fin[0] = finalize
                if deferred_fin[0] is not None:
                    deferred_fin[0]()

    nc.compile()
    _BUILD_CACHE["nc"] = nc
    return nc


def _host_inputs(x, WQ, WK, WV, WO):
    ki = np.arange(128, dtype=np.float32)[:, None]
    qj = np.arange(128, dtype=np.float32)[None, :]
    # stripe mask: within the partial 128-col stripe of diagonal chunk d,
    # allowed iff (qj - 128d) >= ki, i.e. local column >= ki.
    mask = np.where(qj >= ki, 0.0, NEG).astype(np.float32)
    onesb = np.ones((128, 128), dtype=np.float32)

    in_maps = []
    for b in range(B):
        xT = np.ascontiguousarray(x[b].T)
        for g in range(G):
            sl = slice(g * DG, (g + 1) * DG)
            in_maps.append(
                {
                    "xT": xT,
                    "wqT": np.ascontiguousarray(WQ[sl, :].T),
                    "wkT": np.ascontiguousarray(WK[sl, :].T),
                    "wvT": np.ascontiguousarray(WV[sl, :].T),
                    "woT": np.ascontiguousarray(WO[:, sl].T),
                    "mask": mask,
                    "onesb": onesb,
                }
            )
    return in_maps


def kernel(x, WQ, WK, WV, WO):
    from concourse.bass_utils import run_bass_kernel_spmd

    x = np.asarray(x, dtype=np.float32)
    WQ = np.asarray(WQ, dtype=np.float32)
    WK = np.asarray(WK, dtype=np.float32)
    WV = np.asarray(WV, dtype=np.float32)
    WO = np.asarray(WO, dtype=np.float32)

    nc = _build()
    in_maps = _host_inputs(x, WQ, WK, WV, WO)
    res = run_bass_kernel_spmd(
        nc,
        in_maps,
        core_ids=list(range(8)),
        trace=bool(os.environ.get("KERNEL_TRACE")),
    )
    kernel.last_results = res
    parts = [r["out"] for r in res.results]
    out = np.stack([parts[2 * b] + parts[2 * b + 1] for b in range(B)], axis=0)
    return out.astype(np.float32)


# revision 15
# speedup vs baseline: 1.5158x; 1.0127x over previous
"""Causal multi-head self-attention on 8 Trainium2 NeuronCores.

Sharding: core = (batch b, head-group g).  B=4 batches x 2 groups of 8 heads
= 8 cores.  Each core computes Q/K/V projections for its 8 heads, causal
attention, and a partial output projection (row-shard of WO); the host sums
the two partials per batch (the tensor-parallel all-reduce, done at gather).

Per-core device pipeline (all matmuls in float32r = full fp32 precision via
the PE's 2-pass mode, 1 cycle/row at N>=256):
  stage 1: QT[d',s], KT[d',s] (transposed) and V[s,d'] (natural) projections
           from host-pre-transposed xT and weight shards.
  stage 2: q-block outer loop; per (q-block, head-pair): scoresT[k,q] per
           128-wide k-chunk, causal mask add on the single partial 128-col
           stripe of diagonal chunks, exp over the allowed column range only
           (no max subtraction -- scores are O(5) so exp is safe in fp32),
           attn@V with a ones column appended to V so PSUM row 64
           accumulates the softmax denominator.  The kb loop is software
           pipelined (attnV lags scores by one chunk) so the in-order PE
           stream never stalls on the ACT exp.  Denominators gather into
           [128, 512] tiles (rows at pair*32) so reciprocals run 8 rows at
           a time; normalization is an in-place multiply on AT.
  stage 3: output projection for the finished q rows, interleaved with the
           next q-block's attention.
"""

import os
import numpy as np

B, S, D = 4, 2048, 1024
H_TOTAL, DK = 16, 64
G = 2          # head groups (cores per batch)
HG = 8         # heads per core
DG = 512       # head dims per core
CO = 8         # contraction chunks of 128 over D
SBLK = 4       # 512-wide s blocks
QB = 4         # 512-wide q blocks
NEG = -1e9

_BUILD_CACHE = {}


def _build():
    if "nc" in _BUILD_CACHE:
        return _BUILD_CACHE["nc"]

    import concourse.bacc as bacc
    import concourse.mybir as mybir
    import concourse.tile as tile
    from concourse.tile_rust import add_dep_helper

    f32 = mybir.dt.float32
    f32r = mybir.dt.float32r
    AF = mybir.ActivationFunctionType
    ADD = mybir.AluOpType.add
    MULT = mybir.AluOpType.mult

    nc = bacc.Bacc("TRN2", target_bir_lowering=False)
    xT_d = nc.dram_tensor("xT", [D, S], f32, kind="ExternalInput")
    wq_d = nc.dram_tensor("wqT", [D, DG], f32, kind="ExternalInput")
    wk_d = nc.dram_tensor("wkT", [D, DG], f32, kind="ExternalInput")
    wv_d = nc.dram_tensor("wvT", [D, DG], f32, kind="ExternalInput")
    wo_d = nc.dram_tensor("woT", [DG, D], f32, kind="ExternalInput")
    mask_d = nc.dram_tensor("mask", [128, 128], f32, kind="ExternalInput")
    ones_d = nc.dram_tensor("onesb", [128, 128], f32, kind="ExternalInput")
    out_d = nc.dram_tensor("out", [S, D], f32, kind="ExternalOutput")

    with tile.TileContext(nc) as tc:
        with tc.tile_pool(name="persist", bufs=1) as pp:
            QT = pp.tile([128, 4, S], f32r, tag="QT")
            KT = pp.tile([128, 4, S], f32r, tag="KT")
            V = pp.tile([128, 16, HG, DK + 1], f32r, tag="V")
            maskb = pp.tile([128, 128], f32, tag="maskb")
            onesb = pp.tile([128, 128], f32, tag="onesb")
            ones_r = pp.tile([128, 64], f32r, tag="ones_r")
            nc.sync.dma_start(maskb[:], mask_d[:, :])
            nc.sync.dma_start(onesb[:], ones_d[:, :])
            nc.sync.dma_start(ones_r[:], ones_d[:, 0:64].bitcast(f32r))
            # ones column of V (f32 -> f32r rounding copy)
            nc.vector.tensor_copy(
                V[:, :, :, DK : DK + 1],
                onesb[:, 0:128].rearrange("p (so h) -> p so h", so=16)[:, :, :, None],
            )

            # ---------------- stage 1: projections ----------------
            with (
                tc.tile_pool(name="stage1", bufs=1) as s1p,
                tc.tile_pool(name="ps1", bufs=1, space="PSUM") as ps1,
            ):
                wq = s1p.tile([128, CO, DG], f32r, tag="wq")
                wk = s1p.tile([128, CO, DG], f32r, tag="wk")
                wv = s1p.tile([128, CO, DG], f32r, tag="wv")
                nc.sync.dma_start(
                    wq, wq_d[:, :].rearrange("(co ci) d -> ci co d", ci=128).bitcast(f32r)
                )
                nc.sync.dma_start(
                    wk, wk_d[:, :].rearrange("(co ci) d -> ci co d", ci=128).bitcast(f32r)
                )
                nc.sync.dma_start(
                    wv, wv_d[:, :].rearrange("(co ci) d -> ci co d", ci=128).bitcast(f32r)
                )
                for sb in range(SBLK):
                    xt = s1p.tile([128, CO, 512], f32r, tag="xt", bufs=2)
                    nc.sync.dma_start(
                        xt,
                        xT_d[:, sb * 512 : (sb + 1) * 512]
                        .rearrange("(co ci) s -> ci co s", ci=128)
                        .bitcast(f32r),
                    )
                    ssl = slice(sb * 512, (sb + 1) * 512)
                    for do in range(4):
                        dsl = slice(do * 128, (do + 1) * 128)
                        pq = ps1.tile([128, 512], f32, tag="proj", bufs=4, name=f"pq{sb}{do}")
                        for co in range(CO):
                            nc.tensor.matmul(
                                pq, wq[:, co, dsl], xt[:, co, :],
                                start=(co == 0), stop=(co == CO - 1),
                            )
                        nc.any.tensor_copy(QT[:, do, ssl], pq[:])
                        pk = ps1.tile([128, 512], f32, tag="proj", bufs=4, name=f"pk{sb}{do}")
                        for co in range(CO):
                            nc.tensor.matmul(
                                pk, wk[:, co, dsl], xt[:, co, :],
                                start=(co == 0), stop=(co == CO - 1),
                            )
                        nc.any.tensor_copy(KT[:, do, ssl], pk[:])
                    for so in range(4):
                        pv = ps1.tile([128, 512], f32, tag="proj", bufs=4, name=f"pv{sb}{so}")
                        for co in range(CO):
                            nc.tensor.matmul(
                                pv, xt[:, co, so * 128 : (so + 1) * 128], wv[:, co, :],
                                start=(co == 0), stop=(co == CO - 1),
                            )
                        nc.any.tensor_copy(
                            V[:, sb * 4 + so, :, 0:DK],
                            pv[:].rearrange("p (h d) -> p h d", h=HG),
                        )

            # ---------------- stages 2+3: attention + output ----------------
            with (
                tc.tile_pool(name="atp", bufs=1) as atp,
                tc.tile_pool(name="stage2", bufs=1) as s2p,
                tc.tile_pool(name="ps2", bufs=1, space="PSUM") as ps2,
            ):
                AT = atp.tile([128, 4, S], f32r, tag="AT")
                wo = s2p.tile([128, 4, D], f32r, tag="wo")
                nc.sync.dma_start(
                    wo,
                    wo_d[:, :].rearrange("(io ip) j -> ip io j", ip=128).bitcast(f32r),
                )
                pe_prev = [None]  # last instr of the previous PE group

                def pe_group(insts):
                    # force PE issue order: first of this group after last of
                    # the previous group; chain within the group
                    if not insts:
                        return
                    if pe_prev[0] is not None:
                        add_dep_helper(
                            insts[0].ins, pe_prev[0].ins, sync=False,
                            reason="pe group order",
                        )
                    for a, b in zip(insts[1:], insts):
                        add_dep_helper(a.ins, b.ins, sync=False, reason="pe chain")
                    pe_prev[0] = insts[-1]

                deferred_fin = [None]

                for qb in range(QB):
                    qsl = slice(qb * 512, (qb + 1) * 512)
                    nkb = 4 * qb + 4
                    # denominator gather tiles; rows at pair*32 (SBUF
                    # partition offsets must be 32-aligned).  memset to 1.0
                    # so the batched reciprocal sees no garbage lanes.
                    sums_a = s2p.tile([128, 512], f32, tag="sums_a", bufs=2, name=f"sums_a{qb}")
                    sums_b = s2p.tile([128, 512], f32, tag="sums_b", bufs=2, name=f"sums_b{qb}")
                    nc.gpsimd.memset(sums_a[:], 1.0)
                    nc.gpsimd.memset(sums_b[:], 1.0)
                    for pair in range(4):
                        if pair == 1 and deferred_fin[0] is not None:
                            # fire the previous q-block's normalization +
                            # output projection here: pair 0's ov slots are
                            # released and pair 1's not yet allocated, so the
                            # deferred po tiles take clean ovpo ring slots.
                            deferred_fin[0]()
                            deferred_fin[0] = None
                        heads = (2 * pair, 2 * pair + 1)
                        sums_of = {heads[0]: sums_a, heads[1]: sums_b}
                        ovs = {}
                        for h in heads:
                            ov = ps2.tile(
                                [DK + 1, 512], f32, tag="ovpo", bufs=2, name=f"ov{h}q{qb}"
                            )
                            ovs[h] = ov
                        # chunked kb loop with one-chunk lag: emit a
                        # chunk of scores+exps, then the PREVIOUS chunk's
                        # attnV matmuls grouped per head (consecutive
                        # same-bank accumulation -- interleaving score and
                        # attnV matmuls forces a PE pipeline drain at every
                        # lhsT row-group conflict, measured 733 vs 237 ns/mm)
                        def emit_avs(items):
                            grp = []
                            for h in heads:
                                for (pkb, pcs, pets) in items:
                                    grp.append(nc.tensor.matmul(
                                        ovs[h][:, pcs:], V[:, pkb, h, :],
                                        pets[h][:, pcs:],
                                        start=(pkb == 0), stop=(pkb == nkb - 1),
                                    ))
                            pe_group(grp)

                        # balanced chunks of <=3 (avoid tiny trailing
                        # chunks that pay full PE-drain transitions)
                        nch = -(-nkb // 3)
                        lo = nkb // nch
                        hi_cnt = nkb - lo * nch
                        sizes = [lo + 1] * hi_cnt + [lo] * (nch - hi_cnt)
                        kbs = list(range(nkb))
                        chunks, pos = [], 0
                        for sz in sizes:
                            chunks.append(kbs[pos : pos + sz])
                            pos += sz
                        pend = None
                        for chunk in chunks:
                            items = []
                            sc_grp = []
                            for kb in chunk:
                                ksl = slice(kb * 128, (kb + 1) * 128)
                                d = kb - 4 * qb
                                cs = 128 * d if d > 0 else 0
                                ets = {}
                                for h in heads:
                                    base = 64 * (h % 2)
                                    psl = slice(base, base + 64)
                                    sp = ps2.tile(
                                        [128, 512], f32, tag="score", bufs=6,
                                        name=f"sp{h}q{qb}k{kb}",
                                    )
                                    sc_grp.append(nc.tensor.matmul(
                                        sp, KT[psl, pair, ksl], QT[psl, pair, qsl],
                                        start=True, stop=True,
                                    ))
                                    if d >= 0:
                                        nc.vector.tensor_tensor(
                                            sp[:, cs : cs + 128],
                                            sp[:, cs : cs + 128],
                                            maskb[:, :],
                                            ADD,
                                        )
                                    et = s2p.tile(
                                        [128, 512], f32r, tag="et", bufs=12,
                                        name=f"et{h}q{qb}k{kb}",
                                    )
                                    nc.scalar.activation(
                                        et[:, cs:], sp[:, cs:], AF.Exp, scale=0.125
                                    )
                                    ets[h] = et
                                items.append((kb, cs, ets))
                            pe_group(sc_grp)
                            if pend is not None:
                                emit_avs(pend)
                            pend = items
                        emit_avs(pend)
                        for h in heads:
                            base = 64 * (h % 2)
                            ov = ovs[h]
                            # gather denominator row; write unnormalized AT
                            nc.vector.tensor_copy(
                                sums_of[h][pair * 32 : pair * 32 + 1, :],
                                ov[DK : DK + 1, :],
                            )
                            nc.vector.tensor_copy(
                                AT[base : base + 64, pair, qsl], ov[0:DK, :]
                            )
                    # normalize all 8 heads of this q-block: batched
                    # reciprocal, then per head a K=1 broadcast matmul and an
                    # in-place multiply on AT.
                    srec_a = s2p.tile([128, 512], f32r, tag="srec_a", bufs=2, name=f"srec_a{qb}")
                    srec_b = s2p.tile([128, 512], f32r, tag="srec_b", bufs=2, name=f"srec_b{qb}")
                    with nc.allow_low_precision(reason="f32r is fp32-width"):
                        nc.vector.reciprocal(srec_a[:], sums_a[:])
                        nc.vector.reciprocal(srec_b[:], sums_b[:])
                    def finalize(qb=qb, qsl=qsl, srec_a=srec_a, srec_b=srec_b):
                        rbs_ = {}
                        rb_grp = []
                        for pair2 in range(4):
                            heads2 = (2 * pair2, 2 * pair2 + 1)
                            srec_of = {heads2[0]: srec_a, heads2[1]: srec_b}
                            for h in heads2:
                                p32 = pair2 * 32
                                rb = ps2.tile([128, 512], f32, tag="score", bufs=6, name=f"rb{h}{qb}")
                                rb_grp.append(nc.tensor.matmul(
                                    rb[0:64, :],
                                    ones_r[p32 : p32 + 1, :],
                                    srec_of[h][p32 : p32 + 1, :],
                                    start=True, stop=True,
                                    tile_position=(p32, 0) if p32 == 96 else None,
                                ))
                                rbs_[h] = rb
                        pe_group(rb_grp)
                        for pair2 in range(4):
                            for h in (2 * pair2, 2 * pair2 + 1):
                                base = 64 * (h % 2)
                                nc.any.tensor_tensor(
                                    AT[base : base + 64, pair2, qsl],
                                    AT[base : base + 64, pair2, qsl].bitcast(f32),
                                    rbs_[h][0:64, :],
                                    MULT,
                                )
                        for sc in range(4 * qb, 4 * qb + 4):
                            og = s2p.tile([128, D], f32, tag="og", bufs=2, name=f"og{sc}")
                            po_grp = []
                            for jh in range(2):
                                po = ps2.tile(
                                    [128, 512], f32, tag="ovpo", bufs=2, name=f"po{sc}{jh}"
                                )
                                for io in range(4):
                                    po_grp.append(nc.tensor.matmul(
                                        po,
                                        AT[:, io, sc * 128 : (sc + 1) * 128],
                                        wo[:, io, jh * 512 : (jh + 1) * 512],
                                        start=(io == 0), stop=(io == 3),
                                    ))
                                nc.vector.tensor_copy(og[:, jh * 512 : (jh + 1) * 512], po[:])
                            pe_group(po_grp)
                            nc.sync.dma_start(out_d[sc * 128 : (sc + 1) * 128, :], og[:])

                    deferred_fin[0] = finalize
                if deferred_fin[0] is not None:
                    deferred_fin[0]()

    nc.compile()
    _BUILD_CACHE["nc"] = nc
    return nc


def _host_inputs(x, WQ, WK, WV, WO):
    ki = np.arange(128, dtype=np.float32)[:, None]
    qj = np.arange(128, dtype=np.float32)[None, :]
    # stripe mask: within the partial 128-col stripe of diagonal chunk d,
    # allowed iff (qj - 128d) >= ki, i.e. local column >= ki.
    mask = np.where(qj >= ki, 0.0, NEG).astype(np.float32)
    onesb = np.ones((128, 128), dtype=np.float32)

    in_maps = []
    for b in range(B):
        xT = np.ascontiguousarray(x[b].T)
        for g in range(G):
            sl = slice(g * DG, (g + 1) * DG)
            in_maps.append(
                {
                    "xT": xT,
                    "wqT": np.ascontiguousarray(WQ[sl, :].T),
                    "wkT": np.ascontiguousarray(WK[sl, :].T),
                    "wvT": np.ascontiguousarray(WV[sl, :].T),
                    "woT": np.ascontiguousarray(WO[:, sl].T),
                    "mask": mask,
                    "onesb": onesb,
                }
            )
    return in_maps


def kernel(x, WQ, WK, WV, WO):
    from concourse.bass_utils import run_bass_kernel_spmd

    x = np.asarray(x, dtype=np.float32)
    WQ = np.asarray(WQ, dtype=np.float32)
    WK = np.asarray(WK, dtype=np.float32)
    WV = np.asarray(WV, dtype=np.float32)
    WO = np.asarray(WO, dtype=np.float32)

    nc = _build()
    in_maps = _host_inputs(x, WQ, WK, WV, WO)
    res = run_bass_kernel_spmd(
        nc,
        in_maps,
        core_ids=list(range(8)),
        trace=bool(os.environ.get("KERNEL_TRACE")),
    )
    kernel.last_results = res
    parts = [r["out"] for r in res.results]
    out = np.stack([parts[2 * b] + parts[2 * b + 1] for b in range(B)], axis=0)
    return out.astype(np.float32)
